# revision 3
# baseline (speedup 1.0000x reference)
"""EdgeAttentionAggregator Trainium2 kernel (8-core SPMD), v2.

Reference computation (per node n, K=32 neighbors, D=128 out dim, E=64 edge):
    x = features @ W                                    [N, D]
    e[n,k]   = leakyrelu(x[n]@a_self + x[u]@a_nb + emb[n,k]@a_edge),  u=neigh[n,k]
    att      = softmax_k(e)
    h[n]     = sum_k att[n,k] * x[neigh[n,k]]
    h_e[n]   = sum_k att[n,k] * emb[n,k]
    out      = elu([x | h | h_e])                       [N, 2D+E]

Distribution: nodes sharded over 8 cores. Each core projects its shard,
a chunked AllGather replicates a PAIR-row table into every core's DRAM,
and each core resolves its neighbor reads with one dma_gather per tile.

Key hardware-driven choices:
  - dma_gather indices are int16 and elem_size must be a multiple of 256B,
    so the table packs TWO nodes per 512-byte row (25088 rows < 32767).
  - Rotated basis: the table stores y[u] = x[u] @ (Q·S) where Q is orthogonal
    with column 127 = a_nb/|a_nb| and S = diag(1,..,1,|a_nb|). Component 127
    of y IS s_nb = x@a_nb, so no extra s columns ride the row: the pair row
    is exactly [y_even(128 bf16) | y_odd(128 bf16)] = 512B. h is recovered
    from h_tilde = sum att*y by the final PE matmul with M = (Q S^-1)
    (replacing the identity of a plain transpose at zero extra cost).
  - Gather layout is "packed": stream position g*128 + (32*nsub + k) holds
    edge (node 32*nsub + g of the tile, neighbor k). Packed <-> node-major
    is a per-32x32-block transpose = native DVE transpose.
  - h^T on the PE, block g: psum[:, 4g:4g+4] += y_ev_g^T @ A_ev[:, 4g:4g+4]
    + y_od_g^T @ A_od[:, 4g:4g+4]; A_ev/A_od are block-diagonal attention
    matrices masked by parity. h_e^T likewise from packed emb with A (no
    parity split).
  - elu/lrelu affine pieces run on Scalar; the parity-mask products and the
    elu sum on GPSIMD, keeping DVE under the DMA roofline.

Softmax runs without max-subtraction (|logits| < ~40 here, safe in fp32).
elu(v) = relu(v) + exp(-relu(-v)) - 1; lrelu(v) = 0.6v + 0.4|v| (slope 0.2).
"""

import numpy as np
from contextlib import ExitStack

import concourse.bass as bass
import concourse.tile as tile
from concourse import bacc, mybir
from concourse.tile import add_dep_helper
from concourse.bass_utils import run_bass_kernel_spmd
from concourse.masks import make_identity
from concourse import library_config

F32 = mybir.dt.float32
I16 = mybir.dt.int16
BF16 = mybir.dt.bfloat16
AF = mybir.ActivationFunctionType
OP = mybir.AluOpType

ALPHA = 0.2  # leaky relu slope


class Cfg:
    def __init__(self, n_total=50000, k=32, in_dim=256, d=128, e=64, ncores=8,
                 gather_calls=4):  # dma_gather is capped at 1024 idxs/call
        assert n_total % ncores == 0
        assert in_dim % 128 == 0 and d == 128 and k == 32 and e == 64
        self.n_total = n_total
        self.k = k
        self.in_dim = in_dim
        self.d = d
        self.e = e
        self.ncores = ncores
        self.shard = n_total // ncores
        self.tiles = (self.shard + 127) // 128
        self.shard_pad = self.tiles * 128
        self.pairs = self.shard_pad // 2          # pair rows per core
        self.tbl_pairs = ncores * self.pairs
        assert self.tbl_pairs <= 32767
        self.ag_chunks = 7                        # collective split (49 = 7*7)
        assert self.tiles % self.ag_chunks == 0
        self.chunk_pairs = self.pairs // self.ag_chunks
        self.row = 256            # bf16 units per pair row (512 bytes)
        self.wcols = 2 * d + 2    # wext: [y'(D) | x(D) | ssl06 | ssl04]
        self.xcols = d + 2        # xres per tile: [x | ssl06 | ssl04]
        self.out_cols = 2 * d + e
        self.nsub = 128 // k      # 4 nodes per gather block
        self.per_tile_idx = 128 * k
        self.gather_calls = gather_calls
        assert self.per_tile_idx % gather_calls == 0
        self.chunk = self.per_tile_idx // gather_calls   # idxs per call
        self.idx_cols = self.per_tile_idx // 16   # 256 int16 per partition


def build(cfg: Cfg):
    """Build and compile the SPMD Bass module. Returns nc."""
    c = cfg
    nc = bacc.Bacc("TRN2", target_bir_lowering=False, debug=False,
                   num_devices=c.ncores, num_swdge_queues=4)

    featT = nc.dram_tensor("featT", [128, c.tiles * c.in_dim], F32,
                           kind="ExternalInput").ap()
    wext = nc.dram_tensor("wext", [c.in_dim, c.wcols], F32,
                          kind="ExternalInput").ap()
    embd = nc.dram_tensor("embd", [c.tiles * 128, c.k * c.e], BF16,
                          kind="ExternalInput").ap()
    aer = nc.dram_tensor("aer", [128, c.k * c.e], BF16,
                         kind="ExternalInput").ap()
    msk = nc.dram_tensor("msk", [128, 128], BF16, kind="ExternalInput").ap()
    unrot = nc.dram_tensor("unrot", [128, 128], BF16,
                           kind="ExternalInput").ap()
    idx = nc.dram_tensor("idx", [c.tiles * 128, c.idx_cols], I16,
                         kind="ExternalInput").ap()
    parp = nc.dram_tensor("parp", [c.tiles * 128, 2 * c.k], BF16,
                          kind="ExternalInput").ap()
    outd = nc.dram_tensor("outd", [c.tiles * 128, c.out_cols], F32,
                          kind="ExternalOutput").ap()
    shard_pair = nc.dram_tensor("shard_pair", [c.pairs, c.row], BF16).ap()
    table = nc.dram_tensor("table", [c.tbl_pairs, c.row], BF16).ap()

    with tile.TileContext(nc) as tc:
        _body(tc, c, featT, wext, embd, aer, msk, unrot, idx, parp, outd,
              shard_pair, table)

    nc.compile()
    return nc


def _body(tc, c: Cfg, featT, wext, embd, aer, msk, unrot, idx, parp, outd,
          shard_pair, table):
    nc = tc.nc
    D, K, E = c.d, c.k, c.e
    KE = K * E

    with ExitStack() as ctx:
        const = ctx.enter_context(tc.tile_pool(name="const", bufs=1))

        identb = const.tile([128, 128], BF16, tag="identb")
        make_identity(nc, identb[:])

        w_sb = []
        for ci in range(c.in_dim // 128):
            w = const.tile([128, c.wcols], F32, tag=f"w{ci}")
            nc.sync.dma_start(w[:], wext[ci * 128:(ci + 1) * 128, :])
            w_sb.append(w)

        aer_sb = const.tile([128, KE], BF16, tag="aer")
        nc.sync.dma_start(aer_sb[:], aer[:, :])
        msk_sb = const.tile([128, 128], BF16, tag="msk")
        nc.sync.dma_start(msk_sb[:], msk[:, :])
        m_sb = const.tile([128, 128], BF16, tag="m_sb")
        nc.sync.dma_start(m_sb[:], unrot[:, :])

        # resident projected shard (f32): [x | 0.6*s_self | 0.4*s_self]
        xres = const.tile([128, c.tiles * c.xcols], F32, tag="xres")

        # rotating bf16 staging rows (pair halves)
        n_sh = 3
        shtiles = [const.tile([128, D], BF16, tag=f"sh{i}", name=f"sh{i}")
                   for i in range(n_sh)]

        lib = nc.gpsimd.load_library(library_config.mlp)

        # -------- Phase A: project own shard into the rotated basis --------
        shard_writes = []
        with ExitStack() as actx:
            pa = actx.enter_context(tc.tile_pool(name="pa", bufs=3))
            psa = actx.enter_context(
                tc.tile_pool(name="psa", bufs=2, space="PSUM"))
            for t in range(c.tiles):
                ft = pa.tile([128, c.in_dim], F32, tag="ft")
                nc.sync.dma_start(ft[:],
                                  featT[:, t * c.in_dim:(t + 1) * c.in_dim])
                ps = psa.tile([128, c.wcols], F32, tag="ps")
                nchunks = c.in_dim // 128
                for ci in range(nchunks):
                    nc.tensor.matmul(ps[:], lhsT=ft[:, ci * 128:(ci + 1) * 128],
                                     rhs=w_sb[ci][:],
                                     start=(ci == 0), stop=(ci == nchunks - 1))
                nc.scalar.copy(xres[:, t * c.xcols:(t + 1) * c.xcols],
                               ps[:, D:c.wcols])
                sh = shtiles[t % n_sh]
                nc.vector.tensor_copy(sh[:], ps[:, 0:D])
                # write 128 node-rows as 64 pair-rows (parity-major halves)
                wr = nc.sync.dma_start(
                    shard_pair[t * 64:(t + 1) * 64, :]
                    .rearrange("r (p q) -> r p q", p=2),
                    sh[:])
                shard_writes.append(wr)

        # -------- chunked AllGather of the pair-row table --------
        tiles_per_ag = c.tiles // c.ag_chunks
        ccs = []
        for kk in range(c.ag_chunks):
            i0, i1 = kk * c.chunk_pairs, (kk + 1) * c.chunk_pairs
            o0 = kk * c.chunk_pairs * c.ncores
            o1 = (kk + 1) * c.chunk_pairs * c.ncores
            if c.ncores > 1:
                cc = nc.gpsimd.collective_compute(
                    "AllGather", OP.bypass,
                    replica_groups=[list(range(c.ncores))],
                    ins=[shard_pair[i0:i1, :]],
                    outs=[table[o0:o1, :]],
                )
            else:
                cc = nc.sync.dma_start(table[o0:o1, :], shard_pair[i0:i1, :])
            for t in range(kk * tiles_per_ag, (kk + 1) * tiles_per_ag):
                add_dep_helper(cc.ins, shard_writes[t].ins,
                               reason="table chunk after shard write")
            ccs.append(cc)

        # -------- Phase B: attention + aggregation --------
        pb = ctx.enter_context(tc.tile_pool(name="pb", bufs=3))
        pgx = ctx.enter_context(tc.tile_pool(name="pgx", bufs=2))
        psb = ctx.enter_context(tc.tile_pool(name="psb", bufs=2, space="PSUM"))

        for t in range(c.tiles):
            r0, r1 = t * 128, (t + 1) * 128
            idxt = pb.tile([128, c.idx_cols], I16, tag="idxt")
            nc.sync.dma_start(idxt[:], idx[r0:r1, :])
            part = pb.tile([128, 2 * K], BF16, tag="part")
            nc.sync.dma_start(part[:], parp[r0:r1, :])
            embt = pb.tile([128, KE], BF16, tag="embt")
            nc.sync.dma_start(embt[:], embd[r0:r1, :])

            # packed pair-row gather
            gx = pgx.tile([128, K * c.row], BF16, tag="gx")
            nb_per = c.chunk // 128
            for ci in range(c.gather_calls):
                g1 = nc.gpsimd.dma_gather(
                    out_ap=gx[:, ci * nb_per * c.row:(ci + 1) * nb_per * c.row]
                    .rearrange("p (b e) -> p b e", e=c.row),
                    in_ap=table,
                    idxs_ap=idxt[:, ci * (c.chunk // 16):
                                 (ci + 1) * (c.chunk // 16)],
                    num_idxs=c.chunk,
                    num_idxs_reg=c.chunk,
                    elem_size=c.row,
                    queue_num=ci % 4,
                )
                for cc in ccs:
                    add_dep_helper(g1.ins, cc.ins, reason="gather after table")
                add_dep_helper(g1.ins, lib.ins, reason="gather after lib")

            gxv = gx[:].rearrange("p (b q) -> p b q", q=c.row)
            par_pk = part[:, 0:K]       # parity, packed layout
            ipar_pk = part[:, K:2 * K]  # 1 - parity

            # s_nb blend by parity: s = s_ev + par*(s_od - s_ev)
            sdiff = pb.tile([128, K], F32, tag="sdiff")
            nc.vector.tensor_tensor(
                out=sdiff[:].unsqueeze(2), in0=gxv[:, :, c.row - 1:c.row],
                in1=gxv[:, :, D - 1:D], op=OP.subtract)
            sdp = pb.tile([128, K], F32, tag="sdp")
            nc.vector.tensor_tensor(out=sdp[:], in0=sdiff[:], in1=par_pk,
                                    op=OP.mult)
            spk = pb.tile([128, K], F32, tag="spk")
            nc.vector.tensor_tensor(
                out=spk[:].unsqueeze(2), in0=gxv[:, :, D - 1:D],
                in1=sdp[:].unsqueeze(2), op=OP.add)

            # s_edge[n,k] = sum_e emb[n,k,e] * a_edge[e]  (packed layout)
            prod = pb.tile([128, KE], BF16, tag="prod")
            nc.vector.tensor_tensor(out=prod[:], in0=embt[:], in1=aer_sb[:],
                                    op=OP.mult)
            sedge = pb.tile([128, K], F32, tag="sedge")
            nc.vector.tensor_reduce(
                out=sedge[:], in_=prod[:].rearrange("p (k e) -> p k e", k=K),
                axis=mybir.AxisListType.X, op=OP.add)

            etmp = pb.tile([128, K], F32, tag="etmp")
            nc.vector.tensor_tensor(out=etmp[:], in0=spk[:], in1=sedge[:],
                                    op=OP.add)
            etm = pb.tile([128, K], F32, tag="etm")
            nc.vector.transpose(etm[:], etmp[:])   # packed -> node-major

            # e = lrelu(etm + s_self) = 0.6*(etm+s_self) + 0.4*|etm+s_self|
            x0 = t * c.xcols
            ssl06 = xres[:, x0 + D:x0 + D + 1]
            ssl04 = xres[:, x0 + D + 1:x0 + D + 2]
            e6 = pb.tile([128, K], F32, tag="e6")
            nc.scalar.activation(e6[:], etm[:], AF.Identity, bias=ssl06,
                                 scale=1.0 - 2 * ALPHA)
            ab = pb.tile([128, K], F32, tag="ab")
            nc.scalar.activation(ab[:], etm[:], AF.Abs, bias=ssl04,
                                 scale=2 * ALPHA)
            elog = pb.tile([128, K], F32, tag="elog")
            nc.vector.tensor_tensor(out=elog[:], in0=e6[:], in1=ab[:],
                                    op=OP.add)

            # p = exp(e), den = sum_k p (no max-subtraction: |e| small)
            p = pb.tile([128, K], BF16, tag="p")
            den = pb.tile([128, 1], F32, tag="den")
            nc.scalar.activation(p[:], elog[:], AF.Exp, accum_out=den[:])
            inv = pb.tile([128, 1], F32, tag="inv")
            nc.vector.reciprocal(inv[:], den[:])

            # block-diagonal attention, parity-masked
            ppk = pb.tile([128, K], BF16, tag="ppk")
            nc.vector.transpose(ppk[:], p[:])     # node-major -> packed
            asb = pb.tile([128, 128], BF16, tag="asb")
            nc.vector.tensor_tensor(
                out=asb[:],
                in0=ppk[:].unsqueeze(2).to_broadcast([128, K, c.nsub]),
                in1=msk_sb[:], op=OP.mult)
            aev = pb.tile([128, 128], BF16, tag="aev")
            nc.gpsimd.tensor_tensor(
                out=aev[:], in0=asb[:],
                in1=ipar_pk.unsqueeze(2).to_broadcast([128, K, c.nsub]),
                op=OP.mult)
            aod = pb.tile([128, 128], BF16, tag="aod")
            nc.gpsimd.tensor_tensor(
                out=aod[:], in0=asb[:],
                in1=par_pk.unsqueeze(2).to_broadcast([128, K, c.nsub]),
                op=OP.mult)

            # h~^T: per block g accumulate even+odd halves into psum cols
            htps = psb.tile([128, 128], F32, tag="htps")
            for g in range(K):
                nc.tensor.matmul(
                    htps[:, g * c.nsub:(g + 1) * c.nsub],
                    lhsT=gx[:, g * c.row:g * c.row + D],
                    rhs=aev[:, g * c.nsub:(g + 1) * c.nsub],
                    start=True, stop=False)
                nc.tensor.matmul(
                    htps[:, g * c.nsub:(g + 1) * c.nsub],
                    lhsT=gx[:, g * c.row + D:g * c.row + 2 * D],
                    rhs=aod[:, g * c.nsub:(g + 1) * c.nsub],
                    start=False, stop=True)
            # h_e^T from packed emb (no parity split)
            hetps = psb.tile([64, 128], F32, tag="hetps")
            for g in range(K):
                nc.tensor.matmul(
                    hetps[:, g * c.nsub:(g + 1) * c.nsub],
                    lhsT=embt[:, g * E:(g + 1) * E],
                    rhs=asb[:, g * c.nsub:(g + 1) * c.nsub],
                    start=True, stop=True)

            # copy with (g,m)->(m,g) column shuffle so cols become node ids
            htsb = pb.tile([128, 128], BF16, tag="htsb")
            nc.scalar.copy(htsb[:].rearrange("p (m g) -> p m g", m=c.nsub),
                           htps[:].rearrange("p (g m) -> p m g", m=c.nsub))
            hetsb = pb.tile([64, 128], BF16, tag="hetsb")
            nc.scalar.copy(hetsb[:].rearrange("p (m g) -> p m g", m=c.nsub),
                           hetps[:].rearrange("p (g m) -> p m g", m=c.nsub))
            # un-rotate h~ while transposing: hps = htsb^T @ M
            hps = psb.tile([128, D], F32, tag="hps")
            nc.tensor.matmul(hps[:], lhsT=htsb[:], rhs=m_sb[:],
                             start=True, stop=True)
            hets = psb.tile([128, E], BF16, tag="hets")
            nc.tensor.transpose(hets[:], hetsb[:], identb[0:64, 0:64])

            vt = pb.tile([128, c.out_cols], F32, tag="vt")
            nc.scalar.copy(vt[:, 0:D], xres[:, x0:x0 + D])
            nc.scalar.activation(vt[:, D:2 * D], hps[:], AF.Copy, bias=0.0,
                                 scale=inv[:])
            nc.scalar.activation(vt[:, 2 * D:], hets[:], AF.Copy, bias=0.0,
                                 scale=inv[:])

            # elu(v) = relu(v) + exp(-relu(-v)) - 1
            mn = pb.tile([128, c.out_cols], F32, tag="mn")
            nc.scalar.activation(mn[:], vt[:], AF.Relu, scale=-1.0)
            ex = pb.tile([128, c.out_cols], F32, tag="ex")
            nc.scalar.activation(ex[:], mn[:], AF.Exp, scale=-1.0)
            rt = pb.tile([128, c.out_cols], F32, tag="rt")
            nc.scalar.activation(rt[:], vt[:], AF.Relu)
            nc.gpsimd.tensor_tensor(out=vt[:], in0=rt[:], in1=ex[:],
                                    op=OP.add)
            nc.vector.tensor_scalar(out=vt[:], in0=vt[:], scalar1=1.0,
                                    scalar2=None, op0=OP.subtract)

            nc.sync.dma_start(outd[r0:r1, :], vt[:])


# ---------------------------------------------------------------------------
# Host-side driver
# ---------------------------------------------------------------------------

def prep_inputs(cfg: Cfg, features, neigh, emb, W, a):
    """Shard + preprocess full inputs into per-core input maps."""
    import ml_dtypes
    c = cfg
    D, K, E = c.d, c.k, c.e
    a = np.asarray(a, np.float32).reshape(-1)
    a_self, a_nb, a_edge = a[:D], a[D:2 * D], a[2 * D:]
    W = np.asarray(W, np.float32)

    # orthogonal Q with column D-1 = a_nb/|a_nb|; S scales that column
    # back to a_nb so y[:, D-1] = x @ a_nb exactly.
    nrm = float(np.linalg.norm(a_nb))
    rng = np.random.default_rng(0)
    base = rng.standard_normal((D, D))
    base[:, 0] = a_nb / nrm
    Qf, _ = np.linalg.qr(base)
    if np.dot(Qf[:, 0], a_nb) < 0:
        Qf[:, 0] *= -1.0
    Q = np.concatenate([Qf[:, 1:], Qf[:, :1]], axis=1)  # col D-1 = a_nb/nrm
    dscale = np.ones(D, np.float32)
    dscale[D - 1] = nrm
    WQ = (W @ Q) * dscale[None, :]
    ws = W @ a_self
    wext = np.concatenate(
        [WQ, W, ((1.0 - 2 * ALPHA) * ws)[:, None],
         (2 * ALPHA * ws)[:, None]], axis=1)
    wext = np.ascontiguousarray(wext, np.float32)
    # un-rotation matrix: h[d] = sum_y M[y, d] * h~[y]
    m_unrot = np.ascontiguousarray(
        (Q / dscale[None, :]).T.astype(ml_dtypes.bfloat16))

    aer = np.ascontiguousarray(
        np.broadcast_to(np.tile(a_edge, K)[None, :], (128, K * E))
        .astype(ml_dtypes.bfloat16))
    # mask[p, 4g+m] = (p // 32 == m)
    pidx, cidx = np.meshgrid(np.arange(128), np.arange(128), indexing="ij")
    msk_m = ((pidx // K) == (cidx % c.nsub)).astype(ml_dtypes.bfloat16)

    # node u -> (table pair row, parity) under the chunked-AG table layout
    neigh = np.asarray(neigh)
    cidx_n = neigh // c.shard
    j = neigh % c.shard
    lp = j // 2
    par_full = (j & 1).astype(np.float32)
    kk = lp // c.chunk_pairs
    r = lp % c.chunk_pairs
    rowidx = (kk * c.ncores + cidx_n) * c.chunk_pairs + r
    assert rowidx.max() < c.tbl_pairs

    features = np.asarray(features, np.float32)
    emb = np.asarray(emb, np.float32)

    in_maps = []
    for ci in range(c.ncores):
        s0, s1 = ci * c.shard, (ci + 1) * c.shard
        pad = c.shard_pad - c.shard
        f = features[s0:s1]
        if pad:
            f = np.concatenate([f, np.zeros((pad, c.in_dim), np.float32)])
        # host-transposed features: featT[p, (t, c, n)] = f[t*128+n, c*128+p]
        featT = np.ascontiguousarray(
            f.reshape(c.tiles, 128, c.in_dim // 128, 128)
            .transpose(3, 0, 2, 1).reshape(128, c.tiles * c.in_dim))

        em = emb[s0:s1]
        if pad:
            em = np.concatenate([em, np.zeros((pad, K, E), np.float32)])
        # packed emb: embp[t, 32*nsub+k, g*64+e] = em[t, 32*nsub+g, k, e]
        embp = (em.reshape(c.tiles, c.nsub, K, K, E)
                .transpose(0, 1, 3, 2, 4)
                .reshape(c.tiles * 128, K * E).astype(ml_dtypes.bfloat16))

        nr = rowidx[s0:s1]
        pr = par_full[s0:s1]
        if pad:
            nr = np.concatenate([nr, np.zeros((pad, K), nr.dtype)])
            pr = np.concatenate([pr, np.zeros((pad, K), np.float32)])
        # gather stream per tile: pos i = g*128 + (32*nsub + k) holds edge
        # (node 32*nsub + g, neighbor k); block column g = node % 32
        st = (nr.reshape(c.tiles, c.nsub, K, K)    # [t, nsub, g, k]
              .transpose(0, 2, 1, 3)               # [t, g, nsub, k]
              .reshape(c.tiles, c.per_tile_idx)).astype(np.int16)
        # int16 stream wrap per gather call: idx i -> (partition i%16, i//16)
        segs = []
        for cc in range(c.gather_calls):
            seg = st[:, cc * c.chunk:(cc + 1) * c.chunk]
            segs.append(seg.reshape(c.tiles, c.chunk // 16, 16)
                        .transpose(0, 2, 1))       # [t, 16, chunk//16]
        wrapped = np.concatenate(segs, axis=2)     # [t, 16, idx_cols]
        idx16 = np.ascontiguousarray(
            np.tile(wrapped, (1, 8, 1))            # replicate to 128 parts
            .reshape(c.tiles * 128, c.idx_cols))
        # parity in packed layout [p, g]: pos i -> (p=i%128, g=i//128)
        stp = (pr.reshape(c.tiles, c.nsub, K, K)
               .transpose(0, 2, 1, 3).reshape(c.tiles, K, 128))
        par_pk = stp.transpose(0, 2, 1)            # [t, 128, g]
        parr = np.concatenate([par_pk, 1.0 - par_pk], axis=2)
        parr = np.ascontiguousarray(
            parr.reshape(c.tiles * 128, 2 * K).astype(ml_dtypes.bfloat16))
        in_maps.append({
            "featT": featT,
            "wext": wext,
            "embd": np.ascontiguousarray(embp),
            "aer": aer,
            "msk": msk_m,
            "unrot": m_unrot,
            "idx": idx16,
            "parp": parr,
        })
    return in_maps


_CACHE = {}


def _get_compiled(key="full"):
    if key not in _CACHE:
        cfg = Cfg()
        _CACHE[key] = (cfg, build(cfg))
    return _CACHE[key]


def run(inputs, trace=False):
    """Run on hardware. Returns (out [N, 2D+E] f32, exec_time_ns or None)."""
    cfg, nc = _get_compiled()
    in_maps = prep_inputs(cfg, inputs["features"], inputs["neigh"],
                          inputs["emb"], inputs["W"], inputs["a"])
    res = run_bass_kernel_spmd(nc, in_maps, list(range(cfg.ncores)),
                               trace=trace)
    outs = [res.results[ci]["outd"][:cfg.shard] for ci in range(cfg.ncores)]
    out = np.concatenate(outs, axis=0)
    return out, res.exec_time_ns


def kernel(**inputs):
    out, _ = run(inputs)
    return out


# revision 7
# speedup vs baseline: 2.2767x; 2.2767x over previous
"""EdgeAttentionAggregator Trainium2 kernel (8-core SPMD), v2.

Reference computation (per node n, K=32 neighbors, D=128 out dim, E=64 edge):
    x = features @ W                                    [N, D]
    e[n,k]   = leakyrelu(x[n]@a_self + x[u]@a_nb + emb[n,k]@a_edge),  u=neigh[n,k]
    att      = softmax_k(e)
    h[n]     = sum_k att[n,k] * x[neigh[n,k]]
    h_e[n]   = sum_k att[n,k] * emb[n,k]
    out      = elu([x | h | h_e])                       [N, 2D+E]

Distribution: nodes sharded over 8 cores. Each core projects its shard,
a chunked AllGather replicates a PAIR-row table into every core's DRAM,
and each core resolves its neighbor reads with one dma_gather per tile.

Key hardware-driven choices:
  - dma_gather indices are int16 and elem_size must be a multiple of 256B,
    so the table packs TWO nodes per 512-byte row (25088 rows < 32767).
  - Rotated basis: the table stores y[u] = x[u] @ (Q·S) where Q is orthogonal
    with column 127 = a_nb/|a_nb| and S = diag(1,..,1,|a_nb|). Component 127
    of y IS s_nb = x@a_nb, so no extra s columns ride the row: the pair row
    is exactly [y_even(128 bf16) | y_odd(128 bf16)] = 512B. h is recovered
    from h_tilde = sum att*y by the final PE matmul with M = (Q S^-1)
    (replacing the identity of a plain transpose at zero extra cost).
  - Gather layout is "packed": stream position g*128 + (32*nsub + k) holds
    edge (node 32*nsub + g of the tile, neighbor k). Packed <-> node-major
    is a per-32x32-block transpose = native DVE transpose.
  - h^T on the PE, block g: psum[:, 4g:4g+4] += y_ev_g^T @ A_ev[:, 4g:4g+4]
    + y_od_g^T @ A_od[:, 4g:4g+4]; A_ev/A_od are block-diagonal attention
    matrices masked by parity. h_e^T likewise from packed emb with A (no
    parity split).
  - elu/lrelu affine pieces run on Scalar; the parity-mask products and the
    elu sum on GPSIMD, keeping DVE under the DMA roofline.

Softmax runs without max-subtraction (|logits| < ~40 here, safe in fp32).
elu(v) = relu(v) + exp(-relu(-v)) - 1; lrelu(v) = 0.6v + 0.4|v| (slope 0.2).
"""

import numpy as np
from contextlib import ExitStack

import concourse.bass as bass
import concourse.tile as tile
from concourse import bacc, mybir
from concourse.tile import add_dep_helper
from concourse.bass_utils import run_bass_kernel_spmd
from concourse.masks import make_identity
from concourse import library_config

F32 = mybir.dt.float32
I16 = mybir.dt.int16
BF16 = mybir.dt.bfloat16
AF = mybir.ActivationFunctionType
OP = mybir.AluOpType

ALPHA = 0.2  # leaky relu slope


class Cfg:
    def __init__(self, n_total=50000, k=32, in_dim=256, d=128, e=64, ncores=8,
                 gather_calls=4):  # dma_gather is capped at 1024 idxs/call
        assert n_total % ncores == 0
        assert in_dim % 128 == 0 and d == 128 and k == 32 and e == 64
        self.n_total = n_total
        self.k = k
        self.in_dim = in_dim
        self.d = d
        self.e = e
        self.ncores = ncores
        self.shard = n_total // ncores
        self.tiles = (self.shard + 127) // 128
        self.shard_pad = self.tiles * 128
        self.pairs = self.shard_pad // 2          # pair rows per core
        self.tbl_pairs = ncores * self.pairs
        assert self.tbl_pairs <= 32767
        self.ag_chunks = 7                        # collective split (49 = 7*7)
        assert self.tiles % self.ag_chunks == 0
        self.chunk_pairs = self.pairs // self.ag_chunks
        self.row = 256            # bf16 units per pair row (512 bytes)
        self.wcols = 2 * d + 2    # wext: [y'(D) | x(D) | ssl06 | ssl04]
        self.xcols = d + 2        # xres per tile: [x | ssl06 | ssl04]
        self.out_cols = 2 * d + e
        self.nsub = 128 // k      # 4 nodes per gather block
        self.per_tile_idx = 128 * k
        self.gather_calls = gather_calls
        assert self.per_tile_idx % gather_calls == 0
        self.chunk = self.per_tile_idx // gather_calls   # idxs per call
        self.idx_cols = self.per_tile_idx // 16   # 256 int16 per partition


def build(cfg: Cfg):
    """Build and compile the SPMD Bass module. Returns nc."""
    c = cfg
    nc = bacc.Bacc("TRN2", target_bir_lowering=False, debug=False,
                   num_devices=c.ncores, num_swdge_queues=4)

    featT = nc.dram_tensor("featT", [128, c.tiles * c.in_dim], F32,
                           kind="ExternalInput").ap()
    wext = nc.dram_tensor("wext", [c.in_dim, c.wcols], F32,
                          kind="ExternalInput").ap()
    embd = nc.dram_tensor("embd", [c.tiles * 128, c.k * c.e], BF16,
                          kind="ExternalInput").ap()
    aer = nc.dram_tensor("aer", [128, c.k * c.e], BF16,
                         kind="ExternalInput").ap()
    msk = nc.dram_tensor("msk", [128, 128], BF16, kind="ExternalInput").ap()
    unrot = nc.dram_tensor("unrot", [128, 128], BF16,
                           kind="ExternalInput").ap()
    idx = nc.dram_tensor("idx", [c.tiles * 128, c.idx_cols], I16,
                         kind="ExternalInput").ap()
    parp = nc.dram_tensor("parp", [c.tiles * 128, 2 * c.k], BF16,
                          kind="ExternalInput").ap()
    outd = nc.dram_tensor("outd", [c.tiles * 128, c.out_cols], F32,
                          kind="ExternalOutput").ap()
    shard_pair = nc.dram_tensor("shard_pair", [c.pairs, c.row], BF16).ap()
    table = nc.dram_tensor("table", [c.tbl_pairs, c.row], BF16).ap()

    with tile.TileContext(nc) as tc:
        _body(tc, c, featT, wext, embd, aer, msk, unrot, idx, parp, outd,
              shard_pair, table)

    nc.compile()
    return nc


def _body(tc, c: Cfg, featT, wext, embd, aer, msk, unrot, idx, parp, outd,
          shard_pair, table):
    nc = tc.nc
    D, K, E = c.d, c.k, c.e
    KE = K * E

    with ExitStack() as ctx:
        const = ctx.enter_context(tc.tile_pool(name="const", bufs=1))

        identb = const.tile([128, 128], BF16, tag="identb")
        make_identity(nc, identb[:])

        w_sb = []
        for ci in range(c.in_dim // 128):
            w = const.tile([128, c.wcols], F32, tag=f"w{ci}")
            nc.sync.dma_start(w[:], wext[ci * 128:(ci + 1) * 128, :])
            w_sb.append(w)

        aer_sb = const.tile([128, KE], BF16, tag="aer")
        nc.sync.dma_start(aer_sb[:], aer[:, :])
        msk_sb = const.tile([128, 128], BF16, tag="msk")
        nc.sync.dma_start(msk_sb[:], msk[:, :])
        m_sb = const.tile([128, 128], BF16, tag="m_sb")
        nc.sync.dma_start(m_sb[:], unrot[:, :])

        # resident projected shard (f32): [x | 0.6*s_self | 0.4*s_self]
        xres = const.tile([128, c.tiles * c.xcols], F32, tag="xres")

        # rotating bf16 staging rows (pair halves)
        n_sh = 3
        shtiles = [const.tile([128, D], BF16, tag=f"sh{i}", name=f"sh{i}")
                   for i in range(n_sh)]

        lib = nc.gpsimd.load_library(library_config.mlp)

        # -------- Phase A: project own shard into the rotated basis --------
        shard_writes = []
        with ExitStack() as actx:
            pa = actx.enter_context(tc.tile_pool(name="pa", bufs=3))
            psa = actx.enter_context(
                tc.tile_pool(name="psa", bufs=2, space="PSUM"))
            for t in range(c.tiles):
                ft = pa.tile([128, c.in_dim], F32, tag="ft")
                nc.sync.dma_start(ft[:],
                                  featT[:, t * c.in_dim:(t + 1) * c.in_dim])
                ps = psa.tile([128, c.wcols], F32, tag="ps")
                nchunks = c.in_dim // 128
                for ci in range(nchunks):
                    nc.tensor.matmul(ps[:], lhsT=ft[:, ci * 128:(ci + 1) * 128],
                                     rhs=w_sb[ci][:],
                                     start=(ci == 0), stop=(ci == nchunks - 1))
                nc.scalar.copy(xres[:, t * c.xcols:(t + 1) * c.xcols],
                               ps[:, D:c.wcols])
                sh = shtiles[t % n_sh]
                nc.vector.tensor_copy(sh[:], ps[:, 0:D])
                # write 128 node-rows as 64 pair-rows (parity-major halves)
                wr = nc.sync.dma_start(
                    shard_pair[t * 64:(t + 1) * 64, :]
                    .rearrange("r (p q) -> r p q", p=2),
                    sh[:])
                shard_writes.append(wr)

        # -------- AllGather the pair-row table --------
        if c.ncores > 1:
            cc = nc.gpsimd.collective_compute(
                "AllGather", OP.bypass,
                replica_groups=[list(range(c.ncores))],
                ins=[shard_pair[:, :]],
                outs=[table[:, :]],
            )
        else:
            cc = nc.sync.dma_start(table[:, :], shard_pair[:, :])
        for wr in shard_writes:
            add_dep_helper(cc.ins, wr.ins, reason="table after shard write")
        ccs = [cc]

        # -------- Phase B: attention + aggregation --------
        pb = ctx.enter_context(tc.tile_pool(name="pb", bufs=3))
        pgx = ctx.enter_context(tc.tile_pool(name="pgx", bufs=2))
        psb = ctx.enter_context(tc.tile_pool(name="psb", bufs=2, space="PSUM"))

        for t in range(c.tiles):
            r0, r1 = t * 128, (t + 1) * 128
            idxt = pb.tile([128, c.idx_cols], I16, tag="idxt")
            nc.sync.dma_start(idxt[:], idx[r0:r1, :])
            part = pb.tile([128, 2 * K], BF16, tag="part")
            nc.sync.dma_start(part[:], parp[r0:r1, :])
            embt = pb.tile([128, KE], BF16, tag="embt")
            nc.sync.dma_start(embt[:], embd[r0:r1, :])

            # packed pair-row gather
            gx = pgx.tile([128, K * c.row], BF16, tag="gx")
            nb_per = c.chunk // 128
            for ci in range(c.gather_calls):
                g1 = nc.gpsimd.dma_gather(
                    out_ap=gx[:, ci * nb_per * c.row:(ci + 1) * nb_per * c.row]
                    .rearrange("p (b e) -> p b e", e=c.row),
                    in_ap=table,
                    idxs_ap=idxt[:, ci * (c.chunk // 16):
                                 (ci + 1) * (c.chunk // 16)],
                    num_idxs=c.chunk,
                    num_idxs_reg=c.chunk,
                    elem_size=c.row,
                    queue_num=ci % 4,
                )
                for cc in ccs:
                    add_dep_helper(g1.ins, cc.ins, reason="gather after table")
                add_dep_helper(g1.ins, lib.ins, reason="gather after lib")

            gxv = gx[:].rearrange("p (b q) -> p b q", q=c.row)
            par_pk = part[:, 0:K]       # parity, packed layout
            ipar_pk = part[:, K:2 * K]  # 1 - parity

            # s_nb blend by parity: s = s_ev + par*(s_od - s_ev)
            sdiff = pb.tile([128, K], F32, tag="sdiff")
            nc.vector.tensor_tensor(
                out=sdiff[:].unsqueeze(2), in0=gxv[:, :, c.row - 1:c.row],
                in1=gxv[:, :, D - 1:D], op=OP.subtract)
            sdp = pb.tile([128, K], F32, tag="sdp")
            nc.vector.tensor_tensor(out=sdp[:], in0=sdiff[:], in1=par_pk,
                                    op=OP.mult)
            spk = pb.tile([128, K], F32, tag="spk")
            nc.vector.tensor_tensor(
                out=spk[:].unsqueeze(2), in0=gxv[:, :, D - 1:D],
                in1=sdp[:].unsqueeze(2), op=OP.add)

            # s_edge[n,k] = sum_e emb[n,k,e] * a_edge[e]  (packed layout)
            prod = pb.tile([128, KE], BF16, tag="prod")
            nc.vector.tensor_tensor(out=prod[:], in0=embt[:], in1=aer_sb[:],
                                    op=OP.mult)
            sedge = pb.tile([128, K], F32, tag="sedge")
            nc.vector.tensor_reduce(
                out=sedge[:], in_=prod[:].rearrange("p (k e) -> p k e", k=K),
                axis=mybir.AxisListType.X, op=OP.add)

            etmp = pb.tile([128, K], F32, tag="etmp")
            nc.vector.tensor_tensor(out=etmp[:], in0=spk[:], in1=sedge[:],
                                    op=OP.add)
            etm = pb.tile([128, K], F32, tag="etm")
            nc.vector.transpose(etm[:], etmp[:])   # packed -> node-major

            # e = lrelu(etm + s_self) = 0.6*(etm+s_self) + 0.4*|etm+s_self|
            x0 = t * c.xcols
            ssl06 = xres[:, x0 + D:x0 + D + 1]
            ssl04 = xres[:, x0 + D + 1:x0 + D + 2]
            e6 = pb.tile([128, K], F32, tag="e6")
            nc.scalar.activation(e6[:], etm[:], AF.Identity, bias=ssl06,
                                 scale=1.0 - 2 * ALPHA)
            ab = pb.tile([128, K], F32, tag="ab")
            nc.scalar.activation(ab[:], etm[:], AF.Abs, bias=ssl04,
                                 scale=2 * ALPHA)
            elog = pb.tile([128, K], F32, tag="elog")
            nc.vector.tensor_tensor(out=elog[:], in0=e6[:], in1=ab[:],
                                    op=OP.add)

            # p = exp(e), den = sum_k p (no max-subtraction: |e| small)
            p = pb.tile([128, K], BF16, tag="p")
            den = pb.tile([128, 1], F32, tag="den")
            nc.scalar.activation(p[:], elog[:], AF.Exp, accum_out=den[:])
            inv = pb.tile([128, 1], F32, tag="inv")
            nc.vector.reciprocal(inv[:], den[:])

            # block-diagonal attention, parity-masked
            ppk = pb.tile([128, K], BF16, tag="ppk")
            nc.vector.transpose(ppk[:], p[:])     # node-major -> packed
            asb = pb.tile([128, 128], BF16, tag="asb")
            nc.vector.tensor_tensor(
                out=asb[:],
                in0=ppk[:].unsqueeze(2).to_broadcast([128, K, c.nsub]),
                in1=msk_sb[:], op=OP.mult)
            aev = pb.tile([128, 128], BF16, tag="aev")
            nc.vector.tensor_tensor(
                out=aev[:], in0=asb[:],
                in1=ipar_pk.unsqueeze(2).to_broadcast([128, K, c.nsub]),
                op=OP.mult)
            aod = pb.tile([128, 128], BF16, tag="aod")
            nc.vector.tensor_tensor(
                out=aod[:], in0=asb[:],
                in1=par_pk.unsqueeze(2).to_broadcast([128, K, c.nsub]),
                op=OP.mult)

            # h~^T: per block g accumulate even+odd halves into psum cols
            htps = psb.tile([128, 128], F32, tag="htps")
            for g in range(K):
                nc.tensor.matmul(
                    htps[:, g * c.nsub:(g + 1) * c.nsub],
                    lhsT=gx[:, g * c.row:g * c.row + D],
                    rhs=aev[:, g * c.nsub:(g + 1) * c.nsub],
                    start=True, stop=False)
                nc.tensor.matmul(
                    htps[:, g * c.nsub:(g + 1) * c.nsub],
                    lhsT=gx[:, g * c.row + D:g * c.row + 2 * D],
                    rhs=aod[:, g * c.nsub:(g + 1) * c.nsub],
                    start=False, stop=True)
            # h_e^T from packed emb (no parity split)
            hetps = psb.tile([64, 128], F32, tag="hetps")
            for g in range(K):
                nc.tensor.matmul(
                    hetps[:, g * c.nsub:(g + 1) * c.nsub],
                    lhsT=embt[:, g * E:(g + 1) * E],
                    rhs=asb[:, g * c.nsub:(g + 1) * c.nsub],
                    start=True, stop=True)

            # copy with (g,m)->(m,g) column shuffle so cols become node ids
            htsb = pb.tile([128, 128], BF16, tag="htsb")
            nc.scalar.copy(htsb[:].rearrange("p (m g) -> p m g", m=c.nsub),
                           htps[:].rearrange("p (g m) -> p m g", m=c.nsub))
            hetsb = pb.tile([64, 128], BF16, tag="hetsb")
            nc.scalar.copy(hetsb[:].rearrange("p (m g) -> p m g", m=c.nsub),
                           hetps[:].rearrange("p (g m) -> p m g", m=c.nsub))
            # un-rotate h~ while transposing: hps = htsb^T @ M
            hps = psb.tile([128, D], F32, tag="hps")
            nc.tensor.matmul(hps[:], lhsT=htsb[:], rhs=m_sb[:],
                             start=True, stop=True)
            hets = psb.tile([128, E], BF16, tag="hets")
            nc.tensor.transpose(hets[:], hetsb[:], identb[0:64, 0:64])

            vt = pb.tile([128, c.out_cols], F32, tag="vt")
            nc.scalar.copy(vt[:, 0:D], xres[:, x0:x0 + D])
            nc.scalar.activation(vt[:, D:2 * D], hps[:], AF.Copy, bias=0.0,
                                 scale=inv[:])
            nc.scalar.activation(vt[:, 2 * D:], hets[:], AF.Copy, bias=0.0,
                                 scale=inv[:])

            # elu(v) = relu(v) + exp(-relu(-v)) - 1
            mn = pb.tile([128, c.out_cols], F32, tag="mn")
            nc.scalar.activation(mn[:], vt[:], AF.Relu, scale=-1.0)
            ex = pb.tile([128, c.out_cols], F32, tag="ex")
            nc.scalar.activation(ex[:], mn[:], AF.Exp, scale=-1.0)
            rt = pb.tile([128, c.out_cols], F32, tag="rt")
            nc.scalar.activation(rt[:], vt[:], AF.Relu)
            nc.vector.tensor_tensor(out=vt[:], in0=rt[:], in1=ex[:],
                                    op=OP.add)
            nc.vector.tensor_scalar(out=vt[:], in0=vt[:], scalar1=1.0,
                                    scalar2=None, op0=OP.subtract)

            nc.sync.dma_start(outd[r0:r1, :], vt[:])


# ---------------------------------------------------------------------------
# Host-side driver
# ---------------------------------------------------------------------------

def prep_inputs(cfg: Cfg, features, neigh, emb, W, a):
    """Shard + preprocess full inputs into per-core input maps."""
    import ml_dtypes
    c = cfg
    D, K, E = c.d, c.k, c.e
    a = np.asarray(a, np.float32).reshape(-1)
    a_self, a_nb, a_edge = a[:D], a[D:2 * D], a[2 * D:]
    W = np.asarray(W, np.float32)

    # orthogonal Q with column D-1 = a_nb/|a_nb|; S scales that column
    # back to a_nb so y[:, D-1] = x @ a_nb exactly.
    nrm = float(np.linalg.norm(a_nb))
    rng = np.random.default_rng(0)
    base = rng.standard_normal((D, D))
    base[:, 0] = a_nb / nrm
    Qf, _ = np.linalg.qr(base)
    if np.dot(Qf[:, 0], a_nb) < 0:
        Qf[:, 0] *= -1.0
    Q = np.concatenate([Qf[:, 1:], Qf[:, :1]], axis=1)  # col D-1 = a_nb/nrm
    dscale = np.ones(D, np.float32)
    dscale[D - 1] = nrm
    WQ = (W @ Q) * dscale[None, :]
    ws = W @ a_self
    wext = np.concatenate(
        [WQ, W, ((1.0 - 2 * ALPHA) * ws)[:, None],
         (2 * ALPHA * ws)[:, None]], axis=1)
    wext = np.ascontiguousarray(wext, np.float32)
    # un-rotation matrix: h[d] = sum_y M[y, d] * h~[y]
    m_unrot = np.ascontiguousarray(
        (Q / dscale[None, :]).T.astype(ml_dtypes.bfloat16))

    aer = np.ascontiguousarray(
        np.broadcast_to(np.tile(a_edge, K)[None, :], (128, K * E))
        .astype(ml_dtypes.bfloat16))
    # mask[p, 4g+m] = (p // 32 == m)
    pidx, cidx = np.meshgrid(np.arange(128), np.arange(128), indexing="ij")
    msk_m = ((pidx // K) == (cidx % c.nsub)).astype(ml_dtypes.bfloat16)

    # node u -> (table pair row, parity); table rows = per-core shards concat
    neigh = np.asarray(neigh)
    cidx_n = neigh // c.shard
    j = neigh % c.shard
    lp = j // 2
    par_full = (j & 1).astype(np.float32)
    rowidx = cidx_n * c.pairs + lp
    assert rowidx.max() < c.tbl_pairs

    features = np.asarray(features, np.float32)
    emb = np.asarray(emb, np.float32)

    in_maps = []
    for ci in range(c.ncores):
        s0, s1 = ci * c.shard, (ci + 1) * c.shard
        pad = c.shard_pad - c.shard
        f = features[s0:s1]
        if pad:
            f = np.concatenate([f, np.zeros((pad, c.in_dim), np.float32)])
        # host-transposed features: featT[p, (t, c, n)] = f[t*128+n, c*128+p]
        featT = np.ascontiguousarray(
            f.reshape(c.tiles, 128, c.in_dim // 128, 128)
            .transpose(3, 0, 2, 1).reshape(128, c.tiles * c.in_dim))

        em = emb[s0:s1]
        if pad:
            em = np.concatenate([em, np.zeros((pad, K, E), np.float32)])
        # packed emb: embp[t, 32*nsub+k, g*64+e] = em[t, 32*nsub+g, k, e]
        embp = (em.reshape(c.tiles, c.nsub, K, K, E)
                .transpose(0, 1, 3, 2, 4)
                .reshape(c.tiles * 128, K * E).astype(ml_dtypes.bfloat16))

        nr = rowidx[s0:s1]
        pr = par_full[s0:s1]
        if pad:
            nr = np.concatenate([nr, np.zeros((pad, K), nr.dtype)])
            pr = np.concatenate([pr, np.zeros((pad, K), np.float32)])
        # gather stream per tile: pos i = g*128 + (32*nsub + k) holds edge
        # (node 32*nsub + g, neighbor k); block column g = node % 32
        st = (nr.reshape(c.tiles, c.nsub, K, K)    # [t, nsub, g, k]
              .transpose(0, 2, 1, 3)               # [t, g, nsub, k]
              .reshape(c.tiles, c.per_tile_idx)).astype(np.int16)
        # int16 stream wrap per gather call: idx i -> (partition i%16, i//16)
        segs = []
        for cc in range(c.gather_calls):
            seg = st[:, cc * c.chunk:(cc + 1) * c.chunk]
            segs.append(seg.reshape(c.tiles, c.chunk // 16, 16)
                        .transpose(0, 2, 1))       # [t, 16, chunk//16]
        wrapped = np.concatenate(segs, axis=2)     # [t, 16, idx_cols]
        idx16 = np.ascontiguousarray(
            np.tile(wrapped, (1, 8, 1))            # replicate to 128 parts
            .reshape(c.tiles * 128, c.idx_cols))
        # parity in packed layout [p, g]: pos i -> (p=i%128, g=i//128)
        stp = (pr.reshape(c.tiles, c.nsub, K, K)
               .transpose(0, 2, 1, 3).reshape(c.tiles, K, 128))
        par_pk = stp.transpose(0, 2, 1)            # [t, 128, g]
        parr = np.concatenate([par_pk, 1.0 - par_pk], axis=2)
        parr = np.ascontiguousarray(
            parr.reshape(c.tiles * 128, 2 * K).astype(ml_dtypes.bfloat16))
        in_maps.append({
            "featT": featT,
            "wext": wext,
            "embd": np.ascontiguousarray(embp),
            "aer": aer,
            "msk": msk_m,
            "unrot": m_unrot,
            "idx": idx16,
            "parp": parr,
        })
    return in_maps


_CACHE = {}


def _get_compiled(key="full"):
    if key not in _CACHE:
        cfg = Cfg()
        _CACHE[key] = (cfg, build(cfg))
    return _CACHE[key]


def run(inputs, trace=False):
    """Run on hardware. Returns (out [N, 2D+E] f32, exec_time_ns or None)."""
    cfg, nc = _get_compiled()
    in_maps = prep_inputs(cfg, inputs["features"], inputs["neigh"],
                          inputs["emb"], inputs["W"], inputs["a"])
    res = run_bass_kernel_spmd(nc, in_maps, list(range(cfg.ncores)),
                               trace=trace)
    outs = [res.results[ci]["outd"][:cfg.shard] for ci in range(cfg.ncores)]
    out = np.concatenate(outs, axis=0)
    return out, res.exec_time_ns


def kernel(**inputs):
    out, _ = run(inputs)
    return out


# revision 8
# speedup vs baseline: 2.2876x; 1.0048x over previous
"""EdgeAttentionAggregator Trainium2 kernel (8-core SPMD), v2.

Reference computation (per node n, K=32 neighbors, D=128 out dim, E=64 edge):
    x = features @ W                                    [N, D]
    e[n,k]   = leakyrelu(x[n]@a_self + x[u]@a_nb + emb[n,k]@a_edge),  u=neigh[n,k]
    att      = softmax_k(e)
    h[n]     = sum_k att[n,k] * x[neigh[n,k]]
    h_e[n]   = sum_k att[n,k] * emb[n,k]
    out      = elu([x | h | h_e])                       [N, 2D+E]

Distribution: nodes sharded over 8 cores. Each core projects its shard,
a chunked AllGather replicates a PAIR-row table into every core's DRAM,
and each core resolves its neighbor reads with one dma_gather per tile.

Key hardware-driven choices:
  - dma_gather indices are int16 and elem_size must be a multiple of 256B,
    so the table packs TWO nodes per 512-byte row (25088 rows < 32767).
  - Rotated basis: the table stores y[u] = x[u] @ (Q·S) where Q is orthogonal
    with column 127 = a_nb/|a_nb| and S = diag(1,..,1,|a_nb|). Component 127
    of y IS s_nb = x@a_nb, so no extra s columns ride the row: the pair row
    is exactly [y_even(128 bf16) | y_odd(128 bf16)] = 512B. h is recovered
    from h_tilde = sum att*y by the final PE matmul with M = (Q S^-1)
    (replacing the identity of a plain transpose at zero extra cost).
  - Gather layout is "packed": stream position g*128 + (32*nsub + k) holds
    edge (node 32*nsub + g of the tile, neighbor k). Packed <-> node-major
    is a per-32x32-block transpose = native DVE transpose.
  - h^T on the PE, block g: psum[:, 4g:4g+4] += y_ev_g^T @ A_ev[:, 4g:4g+4]
    + y_od_g^T @ A_od[:, 4g:4g+4]; A_ev/A_od are block-diagonal attention
    matrices masked by parity. h_e^T likewise from packed emb with A (no
    parity split).
  - elu/lrelu affine pieces run on Scalar; the parity-mask products and the
    elu sum on GPSIMD, keeping DVE under the DMA roofline.

Softmax runs without max-subtraction (|logits| < ~40 here, safe in fp32).
elu(v) = relu(v) + exp(-relu(-v)) - 1; lrelu(v) = 0.6v + 0.4|v| (slope 0.2).
"""

import numpy as np
from contextlib import ExitStack

import concourse.bass as bass
import concourse.tile as tile
from concourse import bacc, mybir
from concourse.tile import add_dep_helper
from concourse.bass_utils import run_bass_kernel_spmd
from concourse.masks import make_identity
from concourse import library_config

F32 = mybir.dt.float32
I16 = mybir.dt.int16
BF16 = mybir.dt.bfloat16
AF = mybir.ActivationFunctionType
OP = mybir.AluOpType

ALPHA = 0.2  # leaky relu slope


class Cfg:
    def __init__(self, n_total=50000, k=32, in_dim=256, d=128, e=64, ncores=8,
                 gather_calls=4):  # dma_gather is capped at 1024 idxs/call
        assert n_total % ncores == 0
        assert in_dim % 128 == 0 and d == 128 and k == 32 and e == 64
        self.n_total = n_total
        self.k = k
        self.in_dim = in_dim
        self.d = d
        self.e = e
        self.ncores = ncores
        self.shard = n_total // ncores
        self.tiles = (self.shard + 127) // 128
        self.shard_pad = self.tiles * 128
        self.pairs = self.shard_pad // 2          # pair rows per core
        self.tbl_pairs = ncores * self.pairs
        assert self.tbl_pairs <= 32767
        self.ag_chunks = 7                        # collective split (49 = 7*7)
        assert self.tiles % self.ag_chunks == 0
        self.chunk_pairs = self.pairs // self.ag_chunks
        self.row = 256            # bf16 units per pair row (512 bytes)
        self.wcols = 2 * d + 2    # wext: [y'(D) | x(D) | ssl06 | ssl04]
        self.xcols = d + 2        # xres per tile: [x | ssl06 | ssl04]
        self.out_cols = 2 * d + e
        self.nsub = 128 // k      # 4 nodes per gather block
        self.per_tile_idx = 128 * k
        self.gather_calls = gather_calls
        assert self.per_tile_idx % gather_calls == 0
        self.chunk = self.per_tile_idx // gather_calls   # idxs per call
        self.idx_cols = self.per_tile_idx // 16   # 256 int16 per partition


def build(cfg: Cfg):
    """Build and compile the SPMD Bass module. Returns nc."""
    c = cfg
    nc = bacc.Bacc("TRN2", target_bir_lowering=False, debug=False,
                   num_devices=c.ncores, num_swdge_queues=4,
                   dynamic_dma_scratch_size=32768)

    featT = nc.dram_tensor("featT", [128, c.tiles * c.in_dim], F32,
                           kind="ExternalInput").ap()
    wext = nc.dram_tensor("wext", [c.in_dim, c.wcols], F32,
                          kind="ExternalInput").ap()
    embd = nc.dram_tensor("embd", [c.tiles * 128, c.k * c.e], BF16,
                          kind="ExternalInput").ap()
    aer = nc.dram_tensor("aer", [128, c.k * c.e], BF16,
                         kind="ExternalInput").ap()
    msk = nc.dram_tensor("msk", [128, 128], BF16, kind="ExternalInput").ap()
    unrot = nc.dram_tensor("unrot", [128, 128], BF16,
                           kind="ExternalInput").ap()
    idx = nc.dram_tensor("idx", [c.tiles * 128, c.idx_cols], I16,
                         kind="ExternalInput").ap()
    parp = nc.dram_tensor("parp", [c.tiles * 128, 2 * c.k], BF16,
                          kind="ExternalInput").ap()
    outd = nc.dram_tensor("outd", [c.tiles * 128, c.out_cols], F32,
                          kind="ExternalOutput").ap()
    shard_pair = nc.dram_tensor("shard_pair", [c.pairs, c.row], BF16).ap()
    table = nc.dram_tensor("table", [c.tbl_pairs, c.row], BF16).ap()

    with tile.TileContext(nc) as tc:
        _body(tc, c, featT, wext, embd, aer, msk, unrot, idx, parp, outd,
              shard_pair, table)

    nc.compile()
    return nc


def _body(tc, c: Cfg, featT, wext, embd, aer, msk, unrot, idx, parp, outd,
          shard_pair, table):
    nc = tc.nc
    D, K, E = c.d, c.k, c.e
    KE = K * E

    with ExitStack() as ctx:
        const = ctx.enter_context(tc.tile_pool(name="const", bufs=1))

        identb = const.tile([128, 128], BF16, tag="identb")
        make_identity(nc, identb[:])

        w_sb = []
        for ci in range(c.in_dim // 128):
            w = const.tile([128, c.wcols], F32, tag=f"w{ci}")
            nc.sync.dma_start(w[:], wext[ci * 128:(ci + 1) * 128, :])
            w_sb.append(w)

        aer_sb = const.tile([128, KE], BF16, tag="aer")
        nc.sync.dma_start(aer_sb[:], aer[:, :])
        msk_sb = const.tile([128, 128], BF16, tag="msk")
        nc.sync.dma_start(msk_sb[:], msk[:, :])
        m_sb = const.tile([128, 128], BF16, tag="m_sb")
        nc.sync.dma_start(m_sb[:], unrot[:, :])

        # resident projected shard (f32): [x | 0.6*s_self | 0.4*s_self]
        xres = const.tile([128, c.tiles * c.xcols], F32, tag="xres")

        # rotating bf16 staging rows (pair halves)
        n_sh = 3
        shtiles = [const.tile([128, D], BF16, tag=f"sh{i}", name=f"sh{i}")
                   for i in range(n_sh)]

        lib = nc.gpsimd.load_library(library_config.mlp)

        # -------- Phase A: project own shard into the rotated basis --------
        shard_writes = []
        with ExitStack() as actx:
            pa = actx.enter_context(tc.tile_pool(name="pa", bufs=3))
            psa = actx.enter_context(
                tc.tile_pool(name="psa", bufs=2, space="PSUM"))
            for t in range(c.tiles):
                ft = pa.tile([128, c.in_dim], F32, tag="ft")
                nc.sync.dma_start(ft[:],
                                  featT[:, t * c.in_dim:(t + 1) * c.in_dim])
                ps = psa.tile([128, c.wcols], F32, tag="ps")
                nchunks = c.in_dim // 128
                for ci in range(nchunks):
                    nc.tensor.matmul(ps[:], lhsT=ft[:, ci * 128:(ci + 1) * 128],
                                     rhs=w_sb[ci][:],
                                     start=(ci == 0), stop=(ci == nchunks - 1))
                nc.scalar.copy(xres[:, t * c.xcols:(t + 1) * c.xcols],
                               ps[:, D:c.wcols])
                sh = shtiles[t % n_sh]
                nc.vector.tensor_copy(sh[:], ps[:, 0:D])
                # write 128 node-rows as 64 pair-rows (parity-major halves)
                wr = nc.sync.dma_start(
                    shard_pair[t * 64:(t + 1) * 64, :]
                    .rearrange("r (p q) -> r p q", p=2),
                    sh[:])
                shard_writes.append(wr)

        # -------- AllGather the pair-row table --------
        if c.ncores > 1:
            cc = nc.gpsimd.collective_compute(
                "AllGather", OP.bypass,
                replica_groups=[list(range(c.ncores))],
                ins=[shard_pair[:, :]],
                outs=[table[:, :]],
            )
        else:
            cc = nc.sync.dma_start(table[:, :], shard_pair[:, :])
        for wr in shard_writes:
            add_dep_helper(cc.ins, wr.ins, reason="table after shard write")
        ccs = [cc]

        # -------- Phase B: attention + aggregation --------
        pb = ctx.enter_context(tc.tile_pool(name="pb", bufs=3))
        pgx = ctx.enter_context(tc.tile_pool(name="pgx", bufs=2))
        psb = ctx.enter_context(tc.tile_pool(name="psb", bufs=2, space="PSUM"))

        for t in range(c.tiles):
            r0, r1 = t * 128, (t + 1) * 128
            idxt = pb.tile([128, c.idx_cols], I16, tag="idxt")
            nc.sync.dma_start(idxt[:], idx[r0:r1, :])
            part = pb.tile([128, 2 * K], BF16, tag="part")
            nc.sync.dma_start(part[:], parp[r0:r1, :])
            embt = pb.tile([128, KE], BF16, tag="embt")
            nc.sync.dma_start(embt[:], embd[r0:r1, :])

            # packed pair-row gather
            gx = pgx.tile([128, K * c.row], BF16, tag="gx")
            nb_per = c.chunk // 128
            for ci in range(c.gather_calls):
                g1 = nc.gpsimd.dma_gather(
                    out_ap=gx[:, ci * nb_per * c.row:(ci + 1) * nb_per * c.row]
                    .rearrange("p (b e) -> p b e", e=c.row),
                    in_ap=table,
                    idxs_ap=idxt[:, ci * (c.chunk // 16):
                                 (ci + 1) * (c.chunk // 16)],
                    num_idxs=c.chunk,
                    num_idxs_reg=c.chunk,
                    elem_size=c.row,
                    queue_num=ci % 4,
                )
                for cc in ccs:
                    add_dep_helper(g1.ins, cc.ins, reason="gather after table")
                add_dep_helper(g1.ins, lib.ins, reason="gather after lib")

            gxv = gx[:].rearrange("p (b q) -> p b q", q=c.row)
            par_pk = part[:, 0:K]       # parity, packed layout
            ipar_pk = part[:, K:2 * K]  # 1 - parity

            # s_nb blend by parity: s = s_ev + par*(s_od - s_ev)
            sdiff = pb.tile([128, K], F32, tag="sdiff")
            nc.vector.tensor_tensor(
                out=sdiff[:].unsqueeze(2), in0=gxv[:, :, c.row - 1:c.row],
                in1=gxv[:, :, D - 1:D], op=OP.subtract)
            sdp = pb.tile([128, K], F32, tag="sdp")
            nc.vector.tensor_tensor(out=sdp[:], in0=sdiff[:], in1=par_pk,
                                    op=OP.mult)
            spk = pb.tile([128, K], F32, tag="spk")
            nc.vector.tensor_tensor(
                out=spk[:].unsqueeze(2), in0=gxv[:, :, D - 1:D],
                in1=sdp[:].unsqueeze(2), op=OP.add)

            # s_edge[n,k] = sum_e emb[n,k,e] * a_edge[e]  (packed layout)
            prod = pb.tile([128, KE], BF16, tag="prod")
            nc.vector.tensor_tensor(out=prod[:], in0=embt[:], in1=aer_sb[:],
                                    op=OP.mult)
            sedge = pb.tile([128, K], F32, tag="sedge")
            nc.vector.tensor_reduce(
                out=sedge[:], in_=prod[:].rearrange("p (k e) -> p k e", k=K),
                axis=mybir.AxisListType.X, op=OP.add)

            etmp = pb.tile([128, K], F32, tag="etmp")
            nc.vector.tensor_tensor(out=etmp[:], in0=spk[:], in1=sedge[:],
                                    op=OP.add)
            etm = pb.tile([128, K], F32, tag="etm")
            nc.vector.transpose(etm[:], etmp[:])   # packed -> node-major

            # e = lrelu(etm + s_self) = 0.6*(etm+s_self) + 0.4*|etm+s_self|
            x0 = t * c.xcols
            ssl06 = xres[:, x0 + D:x0 + D + 1]
            ssl04 = xres[:, x0 + D + 1:x0 + D + 2]
            e6 = pb.tile([128, K], F32, tag="e6")
            nc.scalar.activation(e6[:], etm[:], AF.Identity, bias=ssl06,
                                 scale=1.0 - 2 * ALPHA)
            ab = pb.tile([128, K], F32, tag="ab")
            nc.scalar.activation(ab[:], etm[:], AF.Abs, bias=ssl04,
                                 scale=2 * ALPHA)
            elog = pb.tile([128, K], F32, tag="elog")
            nc.vector.tensor_tensor(out=elog[:], in0=e6[:], in1=ab[:],
                                    op=OP.add)

            # p = exp(e), den = sum_k p (no max-subtraction: |e| small)
            p = pb.tile([128, K], BF16, tag="p")
            den = pb.tile([128, 1], F32, tag="den")
            nc.scalar.activation(p[:], elog[:], AF.Exp, accum_out=den[:])
            inv = pb.tile([128, 1], F32, tag="inv")
            nc.vector.reciprocal(inv[:], den[:])

            # block-diagonal attention, parity-masked
            ppk = pb.tile([128, K], BF16, tag="ppk")
            nc.vector.transpose(ppk[:], p[:])     # node-major -> packed
            asb = pb.tile([128, 128], BF16, tag="asb")
            nc.vector.tensor_tensor(
                out=asb[:],
                in0=ppk[:].unsqueeze(2).to_broadcast([128, K, c.nsub]),
                in1=msk_sb[:], op=OP.mult)
            aev = pb.tile([128, 128], BF16, tag="aev")
            nc.vector.tensor_tensor(
                out=aev[:], in0=asb[:],
                in1=ipar_pk.unsqueeze(2).to_broadcast([128, K, c.nsub]),
                op=OP.mult)
            aod = pb.tile([128, 128], BF16, tag="aod")
            nc.vector.tensor_tensor(
                out=aod[:], in0=asb[:],
                in1=par_pk.unsqueeze(2).to_broadcast([128, K, c.nsub]),
                op=OP.mult)

            # h~^T: per block g accumulate even+odd halves into psum cols
            htps = psb.tile([128, 128], F32, tag="htps")
            for g in range(K):
                nc.tensor.matmul(
                    htps[:, g * c.nsub:(g + 1) * c.nsub],
                    lhsT=gx[:, g * c.row:g * c.row + D],
                    rhs=aev[:, g * c.nsub:(g + 1) * c.nsub],
                    start=True, stop=False)
                nc.tensor.matmul(
                    htps[:, g * c.nsub:(g + 1) * c.nsub],
                    lhsT=gx[:, g * c.row + D:g * c.row + 2 * D],
                    rhs=aod[:, g * c.nsub:(g + 1) * c.nsub],
                    start=False, stop=True)
            # h_e^T from packed emb (no parity split)
            hetps = psb.tile([64, 128], F32, tag="hetps")
            for g in range(K):
                nc.tensor.matmul(
                    hetps[:, g * c.nsub:(g + 1) * c.nsub],
                    lhsT=embt[:, g * E:(g + 1) * E],
                    rhs=asb[:, g * c.nsub:(g + 1) * c.nsub],
                    start=True, stop=True)

            # copy with (g,m)->(m,g) column shuffle so cols become node ids
            htsb = pb.tile([128, 128], BF16, tag="htsb")
            nc.scalar.copy(htsb[:].rearrange("p (m g) -> p m g", m=c.nsub),
                           htps[:].rearrange("p (g m) -> p m g", m=c.nsub))
            hetsb = pb.tile([64, 128], BF16, tag="hetsb")
            nc.scalar.copy(hetsb[:].rearrange("p (m g) -> p m g", m=c.nsub),
                           hetps[:].rearrange("p (g m) -> p m g", m=c.nsub))
            # un-rotate h~ while transposing: hps = htsb^T @ M
            hps = psb.tile([128, D], F32, tag="hps")
            nc.tensor.matmul(hps[:], lhsT=htsb[:], rhs=m_sb[:],
                             start=True, stop=True)
            hets = psb.tile([128, E], BF16, tag="hets")
            nc.tensor.transpose(hets[:], hetsb[:], identb[0:64, 0:64])

            vt = pb.tile([128, c.out_cols], F32, tag="vt")
            nc.scalar.copy(vt[:, 0:D], xres[:, x0:x0 + D])
            nc.scalar.activation(vt[:, D:2 * D], hps[:], AF.Copy, bias=0.0,
                                 scale=inv[:])
            nc.scalar.activation(vt[:, 2 * D:], hets[:], AF.Copy, bias=0.0,
                                 scale=inv[:])

            # elu(v) = relu(v) + exp(-relu(-v)) - 1
            mn = pb.tile([128, c.out_cols], F32, tag="mn")
            nc.scalar.activation(mn[:], vt[:], AF.Relu, scale=-1.0)
            ex = pb.tile([128, c.out_cols], F32, tag="ex")
            nc.scalar.activation(ex[:], mn[:], AF.Exp, scale=-1.0)
            rt = pb.tile([128, c.out_cols], F32, tag="rt")
            nc.scalar.activation(rt[:], vt[:], AF.Relu)
            nc.vector.tensor_tensor(out=vt[:], in0=rt[:], in1=ex[:],
                                    op=OP.add)
            nc.vector.tensor_scalar(out=vt[:], in0=vt[:], scalar1=1.0,
                                    scalar2=None, op0=OP.subtract)

            nc.sync.dma_start(outd[r0:r1, :], vt[:])


# ---------------------------------------------------------------------------
# Host-side driver
# ---------------------------------------------------------------------------

def prep_inputs(cfg: Cfg, features, neigh, emb, W, a):
    """Shard + preprocess full inputs into per-core input maps."""
    import ml_dtypes
    c = cfg
    D, K, E = c.d, c.k, c.e
    a = np.asarray(a, np.float32).reshape(-1)
    a_self, a_nb, a_edge = a[:D], a[D:2 * D], a[2 * D:]
    W = np.asarray(W, np.float32)

    # orthogonal Q with column D-1 = a_nb/|a_nb|; S scales that column
    # back to a_nb so y[:, D-1] = x @ a_nb exactly.
    nrm = float(np.linalg.norm(a_nb))
    rng = np.random.default_rng(0)
    base = rng.standard_normal((D, D))
    base[:, 0] = a_nb / nrm
    Qf, _ = np.linalg.qr(base)
    if np.dot(Qf[:, 0], a_nb) < 0:
        Qf[:, 0] *= -1.0
    Q = np.concatenate([Qf[:, 1:], Qf[:, :1]], axis=1)  # col D-1 = a_nb/nrm
    dscale = np.ones(D, np.float32)
    dscale[D - 1] = nrm
    WQ = (W @ Q) * dscale[None, :]
    ws = W @ a_self
    wext = np.concatenate(
        [WQ, W, ((1.0 - 2 * ALPHA) * ws)[:, None],
         (2 * ALPHA * ws)[:, None]], axis=1)
    wext = np.ascontiguousarray(wext, np.float32)
    # un-rotation matrix: h[d] = sum_y M[y, d] * h~[y]
    m_unrot = np.ascontiguousarray(
        (Q / dscale[None, :]).T.astype(ml_dtypes.bfloat16))

    aer = np.ascontiguousarray(
        np.broadcast_to(np.tile(a_edge, K)[None, :], (128, K * E))
        .astype(ml_dtypes.bfloat16))
    # mask[p, 4g+m] = (p // 32 == m)
    pidx, cidx = np.meshgrid(np.arange(128), np.arange(128), indexing="ij")
    msk_m = ((pidx // K) == (cidx % c.nsub)).astype(ml_dtypes.bfloat16)

    # node u -> (table pair row, parity); table rows = per-core shards concat
    neigh = np.asarray(neigh)
    cidx_n = neigh // c.shard
    j = neigh % c.shard
    lp = j // 2
    par_full = (j & 1).astype(np.float32)
    rowidx = cidx_n * c.pairs + lp
    assert rowidx.max() < c.tbl_pairs

    features = np.asarray(features, np.float32)
    emb = np.asarray(emb, np.float32)

    in_maps = []
    for ci in range(c.ncores):
        s0, s1 = ci * c.shard, (ci + 1) * c.shard
        pad = c.shard_pad - c.shard
        f = features[s0:s1]
        if pad:
            f = np.concatenate([f, np.zeros((pad, c.in_dim), np.float32)])
        # host-transposed features: featT[p, (t, c, n)] = f[t*128+n, c*128+p]
        featT = np.ascontiguousarray(
            f.reshape(c.tiles, 128, c.in_dim // 128, 128)
            .transpose(3, 0, 2, 1).reshape(128, c.tiles * c.in_dim))

        em = emb[s0:s1]
        if pad:
            em = np.concatenate([em, np.zeros((pad, K, E), np.float32)])
        # packed emb: embp[t, 32*nsub+k, g*64+e] = em[t, 32*nsub+g, k, e]
        embp = (em.reshape(c.tiles, c.nsub, K, K, E)
                .transpose(0, 1, 3, 2, 4)
                .reshape(c.tiles * 128, K * E).astype(ml_dtypes.bfloat16))

        nr = rowidx[s0:s1]
        pr = par_full[s0:s1]
        if pad:
            nr = np.concatenate([nr, np.zeros((pad, K), nr.dtype)])
            pr = np.concatenate([pr, np.zeros((pad, K), np.float32)])
        # gather stream per tile: pos i = g*128 + (32*nsub + k) holds edge
        # (node 32*nsub + g, neighbor k); block column g = node % 32
        st = (nr.reshape(c.tiles, c.nsub, K, K)    # [t, nsub, g, k]
              .transpose(0, 2, 1, 3)               # [t, g, nsub, k]
              .reshape(c.tiles, c.per_tile_idx)).astype(np.int16)
        # int16 stream wrap per gather call: idx i -> (partition i%16, i//16)
        segs = []
        for cc in range(c.gather_calls):
            seg = st[:, cc * c.chunk:(cc + 1) * c.chunk]
            segs.append(seg.reshape(c.tiles, c.chunk // 16, 16)
                        .transpose(0, 2, 1))       # [t, 16, chunk//16]
        wrapped = np.concatenate(segs, axis=2)     # [t, 16, idx_cols]
        idx16 = np.ascontiguousarray(
            np.tile(wrapped, (1, 8, 1))            # replicate to 128 parts
            .reshape(c.tiles * 128, c.idx_cols))
        # parity in packed layout [p, g]: pos i -> (p=i%128, g=i//128)
        stp = (pr.reshape(c.tiles, c.nsub, K, K)
               .transpose(0, 2, 1, 3).reshape(c.tiles, K, 128))
        par_pk = stp.transpose(0, 2, 1)            # [t, 128, g]
        parr = np.concatenate([par_pk, 1.0 - par_pk], axis=2)
        parr = np.ascontiguousarray(
            parr.reshape(c.tiles * 128, 2 * K).astype(ml_dtypes.bfloat16))
        in_maps.append({
            "featT": featT,
            "wext": wext,
            "embd": np.ascontiguousarray(embp),
            "aer": aer,
            "msk": msk_m,
            "unrot": m_unrot,
            "idx": idx16,
            "parp": parr,
        })
    return in_maps


_CACHE = {}


def _get_compiled(key="full"):
    if key not in _CACHE:
        cfg = Cfg()
        _CACHE[key] = (cfg, build(cfg))
    return _CACHE[key]


def run(inputs, trace=False):
    """Run on hardware. Returns (out [N, 2D+E] f32, exec_time_ns or None)."""
    cfg, nc = _get_compiled()
    in_maps = prep_inputs(cfg, inputs["features"], inputs["neigh"],
                          inputs["emb"], inputs["W"], inputs["a"])
    res = run_bass_kernel_spmd(nc, in_maps, list(range(cfg.ncores)),
                               trace=trace)
    outs = [res.results[ci]["outd"][:cfg.shard] for ci in range(cfg.ncores)]
    out = np.concatenate(outs, axis=0)
    return out, res.exec_time_ns


def kernel(**inputs):
    out, _ = run(inputs)
    return out


# revision 10
# speedup vs baseline: 2.7752x; 1.2132x over previous
"""EdgeAttentionAggregator Trainium2 kernel (8-core SPMD), v2.

Reference computation (per node n, K=32 neighbors, D=128 out dim, E=64 edge):
    x = features @ W                                    [N, D]
    e[n,k]   = leakyrelu(x[n]@a_self + x[u]@a_nb + emb[n,k]@a_edge),  u=neigh[n,k]
    att      = softmax_k(e)
    h[n]     = sum_k att[n,k] * x[neigh[n,k]]
    h_e[n]   = sum_k att[n,k] * emb[n,k]
    out      = elu([x | h | h_e])                       [N, 2D+E]

Distribution: nodes sharded over 8 cores. Each core projects its shard,
a chunked AllGather replicates a PAIR-row table into every core's DRAM,
and each core resolves its neighbor reads with one dma_gather per tile.

Key hardware-driven choices:
  - dma_gather indices are int16 and elem_size must be a multiple of 256B,
    so the table packs TWO nodes per 512-byte row (25088 rows < 32767).
  - Rotated basis: the table stores y[u] = x[u] @ (Q·S) where Q is orthogonal
    with column 127 = a_nb/|a_nb| and S = diag(1,..,1,|a_nb|). Component 127
    of y IS s_nb = x@a_nb, so no extra s columns ride the row: the pair row
    is exactly [y_even(128 bf16) | y_odd(128 bf16)] = 512B. h is recovered
    from h_tilde = sum att*y by the final PE matmul with M = (Q S^-1)
    (replacing the identity of a plain transpose at zero extra cost).
  - Gather layout is "packed": stream position g*128 + (32*nsub + k) holds
    edge (node 32*nsub + g of the tile, neighbor k). Packed <-> node-major
    is a per-32x32-block transpose = native DVE transpose.
  - h^T on the PE, block g: psum[:, 4g:4g+4] += y_ev_g^T @ A_ev[:, 4g:4g+4]
    + y_od_g^T @ A_od[:, 4g:4g+4]; A_ev/A_od are block-diagonal attention
    matrices masked by parity. h_e^T likewise from packed emb with A (no
    parity split).
  - elu/lrelu affine pieces run on Scalar; the parity-mask products and the
    elu sum on GPSIMD, keeping DVE under the DMA roofline.

Softmax runs without max-subtraction (|logits| < ~40 here, safe in fp32).
elu(v) = relu(v) + exp(-relu(-v)) - 1; lrelu(v) = 0.6v + 0.4|v| (slope 0.2).
"""

import numpy as np
from contextlib import ExitStack

import concourse.bass as bass
import concourse.tile as tile
from concourse import bacc, mybir
from concourse.tile import add_dep_helper
from concourse.bass_utils import run_bass_kernel_spmd
from concourse.masks import make_identity
from concourse import library_config

F32 = mybir.dt.float32
I16 = mybir.dt.int16
BF16 = mybir.dt.bfloat16
AF = mybir.ActivationFunctionType
OP = mybir.AluOpType

ALPHA = 0.2  # leaky relu slope


class Cfg:
    def __init__(self, n_total=50000, k=32, in_dim=256, d=128, e=64, ncores=8,
                 gather_calls=4):  # dma_gather is capped at 1024 idxs/call
        assert n_total % ncores == 0
        assert in_dim % 128 == 0 and d == 128 and k == 32 and e == 64
        self.n_total = n_total
        self.k = k
        self.in_dim = in_dim
        self.d = d
        self.e = e
        self.ncores = ncores
        self.shard = n_total // ncores
        self.tiles = (self.shard + 127) // 128
        self.shard_pad = self.tiles * 128
        self.pairs = self.shard_pad // 2          # pair rows per core
        self.tbl_pairs = ncores * self.pairs
        assert self.tbl_pairs <= 32767
        self.ag_chunks = 7                        # collective split (49 = 7*7)
        assert self.tiles % self.ag_chunks == 0
        self.chunk_pairs = self.pairs // self.ag_chunks
        self.row = 256            # bf16 units per pair row (512 bytes)
        self.wcols = 2 * d + 2    # wext: [y'(D) | x(D) | ssl06 | ssl04]
        self.xcols = d + 2        # xres per tile: [x | ssl06 | ssl04]
        self.out_cols = 2 * d + e
        self.nsub = 128 // k      # 4 nodes per gather block
        self.per_tile_idx = 128 * k
        self.gather_calls = gather_calls
        assert self.per_tile_idx % gather_calls == 0
        self.chunk = self.per_tile_idx // gather_calls   # idxs per call
        self.idx_cols = self.per_tile_idx // 16   # 256 int16 per partition


def build(cfg: Cfg):
    """Build and compile the SPMD Bass module. Returns nc."""
    c = cfg
    nc = bacc.Bacc("TRN2", target_bir_lowering=False, debug=False,
                   num_devices=c.ncores, num_swdge_queues=4,
                   dynamic_dma_scratch_size=32768)

    featT = nc.dram_tensor("featT", [128, c.tiles * c.in_dim], F32,
                           kind="ExternalInput").ap()
    wext = nc.dram_tensor("wext", [c.in_dim, c.wcols], F32,
                          kind="ExternalInput").ap()
    embd = nc.dram_tensor("embd", [c.tiles * 128, c.k * c.e], BF16,
                          kind="ExternalInput").ap()
    aer = nc.dram_tensor("aer", [128, c.k * c.e], BF16,
                         kind="ExternalInput").ap()
    msk = nc.dram_tensor("msk", [128, 128], BF16, kind="ExternalInput").ap()
    unrot = nc.dram_tensor("unrot", [128, 128], BF16,
                           kind="ExternalInput").ap()
    idx = nc.dram_tensor("idx", [c.tiles * 128, c.idx_cols], I16,
                         kind="ExternalInput").ap()
    parp = nc.dram_tensor("parp", [c.tiles * 128, 2 * c.k], BF16,
                          kind="ExternalInput").ap()
    outd = nc.dram_tensor("outd", [c.tiles * 128, c.out_cols], F32,
                          kind="ExternalOutput").ap()
    shard_pair = nc.dram_tensor("shard_pair", [c.pairs, c.row], BF16).ap()
    table = nc.dram_tensor("table", [c.tbl_pairs, c.row], BF16).ap()

    with tile.TileContext(nc) as tc:
        _body(tc, c, featT, wext, embd, aer, msk, unrot, idx, parp, outd,
              shard_pair, table)

    nc.compile()
    return nc


def _body(tc, c: Cfg, featT, wext, embd, aer, msk, unrot, idx, parp, outd,
          shard_pair, table):
    nc = tc.nc
    D, K, E = c.d, c.k, c.e
    KE = K * E

    with ExitStack() as ctx:
        const = ctx.enter_context(tc.tile_pool(name="const", bufs=1))

        identb = const.tile([128, 128], BF16, tag="identb")
        make_identity(nc, identb[:])

        w_sb = []
        for ci in range(c.in_dim // 128):
            w = const.tile([128, c.wcols], F32, tag=f"w{ci}")
            nc.sync.dma_start(w[:], wext[ci * 128:(ci + 1) * 128, :])
            w_sb.append(w)

        aer_sb = const.tile([128, KE], BF16, tag="aer")
        nc.sync.dma_start(aer_sb[:], aer[:, :])
        msk_sb = const.tile([128, 128], BF16, tag="msk")
        nc.sync.dma_start(msk_sb[:], msk[:, :])
        m_sb = const.tile([128, 128], BF16, tag="m_sb")
        nc.sync.dma_start(m_sb[:], unrot[:, :])

        # resident projected shard (f32): [x | 0.6*s_self | 0.4*s_self]
        xres = const.tile([128, c.tiles * c.xcols], F32, tag="xres")

        # rotating bf16 staging rows (pair halves)
        n_sh = 3
        shtiles = [const.tile([128, D], BF16, tag=f"sh{i}", name=f"sh{i}")
                   for i in range(n_sh)]

        lib = nc.gpsimd.load_library(library_config.mlp)

        # -------- Phase A: project own shard into the rotated basis --------
        shard_writes = []
        with ExitStack() as actx:
            pa = actx.enter_context(tc.tile_pool(name="pa", bufs=3))
            psa = actx.enter_context(
                tc.tile_pool(name="psa", bufs=2, space="PSUM"))
            for t in range(c.tiles):
                ft = pa.tile([128, c.in_dim], F32, tag="ft")
                nc.sync.dma_start(ft[:],
                                  featT[:, t * c.in_dim:(t + 1) * c.in_dim])
                ps = psa.tile([128, c.wcols], F32, tag="ps")
                nchunks = c.in_dim // 128
                for ci in range(nchunks):
                    nc.tensor.matmul(ps[:], lhsT=ft[:, ci * 128:(ci + 1) * 128],
                                     rhs=w_sb[ci][:],
                                     start=(ci == 0), stop=(ci == nchunks - 1))
                nc.scalar.copy(xres[:, t * c.xcols:(t + 1) * c.xcols],
                               ps[:, D:c.wcols])
                sh = shtiles[t % n_sh]
                nc.vector.tensor_copy(sh[:], ps[:, 0:D])
                # write 128 node-rows as 64 pair-rows (parity-major halves)
                wr = nc.sync.dma_start(
                    shard_pair[t * 64:(t + 1) * 64, :]
                    .rearrange("r (p q) -> r p q", p=2),
                    sh[:])
                shard_writes.append(wr)

        # -------- AllGather the pair-row table --------
        if c.ncores > 1:
            cc = nc.gpsimd.collective_compute(
                "AllGather", OP.bypass,
                replica_groups=[list(range(c.ncores))],
                ins=[shard_pair[:, :]],
                outs=[table[:, :]],
            )
        else:
            cc = nc.sync.dma_start(table[:, :], shard_pair[:, :])
        for wr in shard_writes:
            add_dep_helper(cc.ins, wr.ins, reason="table after shard write")
        ccs = [cc]

        # -------- Phase B: attention + aggregation --------
        # Software-pipelined: stage(t) issues input DMAs + gather dispatch
        # DEPTH tiles ahead of compute(t) so the gather DMA latency and the
        # downstream engine queues fully overlap across tiles.
        pst = ctx.enter_context(tc.tile_pool(name="pst", bufs=3))
        pb = ctx.enter_context(tc.tile_pool(name="pb", bufs=2))
        pgx = ctx.enter_context(tc.tile_pool(name="pgx", bufs=3))
        psb = ctx.enter_context(tc.tile_pool(name="psb", bufs=2, space="PSUM"))

        def stage(t):
            r0, r1 = t * 128, (t + 1) * 128
            idxt = pst.tile([128, c.idx_cols], I16, tag="idxt")
            nc.sync.dma_start(idxt[:], idx[r0:r1, :])
            part = pst.tile([128, 2 * K], BF16, tag="part")
            nc.sync.dma_start(part[:], parp[r0:r1, :])
            embt = pst.tile([128, KE], BF16, tag="embt")
            nc.sync.dma_start(embt[:], embd[r0:r1, :])
            gx = pgx.tile([128, K * c.row], BF16, tag="gx")
            nb_per = c.chunk // 128
            for ci in range(c.gather_calls):
                g1 = nc.gpsimd.dma_gather(
                    out_ap=gx[:, ci * nb_per * c.row:(ci + 1) * nb_per * c.row]
                    .rearrange("p (b e) -> p b e", e=c.row),
                    in_ap=table,
                    idxs_ap=idxt[:, ci * (c.chunk // 16):
                                 (ci + 1) * (c.chunk // 16)],
                    num_idxs=c.chunk,
                    num_idxs_reg=c.chunk,
                    elem_size=c.row,
                    queue_num=ci % 4,
                )
                for cc in ccs:
                    add_dep_helper(g1.ins, cc.ins, reason="gather after table")
                add_dep_helper(g1.ins, lib.ins, reason="gather after lib")
            return idxt, part, embt, gx

        def compute(t, st):
            r0, r1 = t * 128, (t + 1) * 128
            idxt, part, embt, gx = st
            gxv = gx[:].rearrange("p (b q) -> p b q", q=c.row)
            par_pk = part[:, 0:K]       # parity, packed layout
            ipar_pk = part[:, K:2 * K]  # 1 - parity

            # s_nb blend by parity: s = s_ev + par*(s_od - s_ev)
            sdiff = pb.tile([128, K], F32, tag="sdiff")
            nc.vector.tensor_tensor(
                out=sdiff[:].unsqueeze(2), in0=gxv[:, :, c.row - 1:c.row],
                in1=gxv[:, :, D - 1:D], op=OP.subtract)
            sdp = pb.tile([128, K], F32, tag="sdp")
            nc.vector.tensor_tensor(out=sdp[:], in0=sdiff[:], in1=par_pk,
                                    op=OP.mult)
            spk = pb.tile([128, K], F32, tag="spk")
            nc.vector.tensor_tensor(
                out=spk[:].unsqueeze(2), in0=gxv[:, :, D - 1:D],
                in1=sdp[:].unsqueeze(2), op=OP.add)

            # s_edge[n,k] = sum_e emb[n,k,e] * a_edge[e]  (packed layout)
            prod = pb.tile([128, KE], BF16, tag="prod")
            nc.vector.tensor_tensor(out=prod[:], in0=embt[:], in1=aer_sb[:],
                                    op=OP.mult)
            sedge = pb.tile([128, K], F32, tag="sedge")
            nc.vector.tensor_reduce(
                out=sedge[:], in_=prod[:].rearrange("p (k e) -> p k e", k=K),
                axis=mybir.AxisListType.X, op=OP.add)

            etmp = pb.tile([128, K], F32, tag="etmp")
            nc.vector.tensor_tensor(out=etmp[:], in0=spk[:], in1=sedge[:],
                                    op=OP.add)
            etm = pb.tile([128, K], F32, tag="etm")
            nc.vector.transpose(etm[:], etmp[:])   # packed -> node-major

            # e = lrelu(etm + s_self) = 0.6*(etm+s_self) + 0.4*|etm+s_self|
            x0 = t * c.xcols
            ssl06 = xres[:, x0 + D:x0 + D + 1]
            ssl04 = xres[:, x0 + D + 1:x0 + D + 2]
            e6 = pb.tile([128, K], F32, tag="e6")
            nc.scalar.activation(e6[:], etm[:], AF.Identity, bias=ssl06,
                                 scale=1.0 - 2 * ALPHA)
            ab = pb.tile([128, K], F32, tag="ab")
            nc.scalar.activation(ab[:], etm[:], AF.Abs, bias=ssl04,
                                 scale=2 * ALPHA)
            elog = pb.tile([128, K], F32, tag="elog")
            nc.vector.tensor_tensor(out=elog[:], in0=e6[:], in1=ab[:],
                                    op=OP.add)

            # p = exp(e), den = sum_k p (no max-subtraction: |e| small)
            p = pb.tile([128, K], BF16, tag="p")
            den = pb.tile([128, 1], F32, tag="den")
            nc.scalar.activation(p[:], elog[:], AF.Exp, accum_out=den[:])
            inv = pb.tile([128, 1], F32, tag="inv")
            nc.vector.reciprocal(inv[:], den[:])

            # block-diagonal attention, parity-masked
            ppk = pb.tile([128, K], BF16, tag="ppk")
            nc.vector.transpose(ppk[:], p[:])     # node-major -> packed
            asb = pb.tile([128, 128], BF16, tag="asb")
            nc.vector.tensor_tensor(
                out=asb[:],
                in0=ppk[:].unsqueeze(2).to_broadcast([128, K, c.nsub]),
                in1=msk_sb[:], op=OP.mult)
            aev = pb.tile([128, 128], BF16, tag="aev")
            nc.vector.tensor_tensor(
                out=aev[:], in0=asb[:],
                in1=ipar_pk.unsqueeze(2).to_broadcast([128, K, c.nsub]),
                op=OP.mult)
            aod = pb.tile([128, 128], BF16, tag="aod")
            nc.vector.tensor_tensor(
                out=aod[:], in0=asb[:],
                in1=par_pk.unsqueeze(2).to_broadcast([128, K, c.nsub]),
                op=OP.mult)

            # h~^T: per block g accumulate even+odd halves into psum cols
            htps = psb.tile([128, 128], F32, tag="htps")
            for g in range(K):
                nc.tensor.matmul(
                    htps[:, g * c.nsub:(g + 1) * c.nsub],
                    lhsT=gx[:, g * c.row:g * c.row + D],
                    rhs=aev[:, g * c.nsub:(g + 1) * c.nsub],
                    start=True, stop=False)
                nc.tensor.matmul(
                    htps[:, g * c.nsub:(g + 1) * c.nsub],
                    lhsT=gx[:, g * c.row + D:g * c.row + 2 * D],
                    rhs=aod[:, g * c.nsub:(g + 1) * c.nsub],
                    start=False, stop=True)
            # h_e^T from packed emb (no parity split)
            hetps = psb.tile([64, 128], F32, tag="hetps")
            for g in range(K):
                nc.tensor.matmul(
                    hetps[:, g * c.nsub:(g + 1) * c.nsub],
                    lhsT=embt[:, g * E:(g + 1) * E],
                    rhs=asb[:, g * c.nsub:(g + 1) * c.nsub],
                    start=True, stop=True)

            # copy with (g,m)->(m,g) column shuffle so cols become node ids
            htsb = pb.tile([128, 128], BF16, tag="htsb")
            nc.scalar.copy(htsb[:].rearrange("p (m g) -> p m g", m=c.nsub),
                           htps[:].rearrange("p (g m) -> p m g", m=c.nsub))
            hetsb = pb.tile([64, 128], BF16, tag="hetsb")
            nc.scalar.copy(hetsb[:].rearrange("p (m g) -> p m g", m=c.nsub),
                           hetps[:].rearrange("p (g m) -> p m g", m=c.nsub))
            # un-rotate h~ while transposing: hps = htsb^T @ M
            hps = psb.tile([128, D], F32, tag="hps")
            nc.tensor.matmul(hps[:], lhsT=htsb[:], rhs=m_sb[:],
                             start=True, stop=True)
            hets = psb.tile([128, E], BF16, tag="hets")
            nc.tensor.transpose(hets[:], hetsb[:], identb[0:64, 0:64])

            vt = pb.tile([128, c.out_cols], F32, tag="vt")
            nc.scalar.copy(vt[:, 0:D], xres[:, x0:x0 + D])
            nc.scalar.activation(vt[:, D:2 * D], hps[:], AF.Copy, bias=0.0,
                                 scale=inv[:])
            nc.scalar.activation(vt[:, 2 * D:], hets[:], AF.Copy, bias=0.0,
                                 scale=inv[:])

            # elu(v) = relu(v) + exp(-relu(-v)) - 1
            mn = pb.tile([128, c.out_cols], F32, tag="mn")
            nc.scalar.activation(mn[:], vt[:], AF.Relu, scale=-1.0)
            ex = pb.tile([128, c.out_cols], F32, tag="ex")
            nc.scalar.activation(ex[:], mn[:], AF.Exp, scale=-1.0)
            rt = pb.tile([128, c.out_cols], F32, tag="rt")
            nc.scalar.activation(rt[:], vt[:], AF.Relu)
            nc.vector.tensor_tensor(out=vt[:], in0=rt[:], in1=ex[:],
                                    op=OP.add)
            nc.vector.tensor_scalar(out=vt[:], in0=vt[:], scalar1=1.0,
                                    scalar2=None, op0=OP.subtract)

            nc.sync.dma_start(outd[r0:r1, :], vt[:])

        depth = 1
        staged = {}
        for t in range(min(depth + 1, c.tiles)):
            staged[t] = stage(t)
        for t in range(c.tiles):
            compute(t, staged.pop(t))
            nt = t + depth + 1
            if nt < c.tiles:
                staged[nt] = stage(nt)


# ---------------------------------------------------------------------------
# Host-side driver
# ---------------------------------------------------------------------------

def prep_inputs(cfg: Cfg, features, neigh, emb, W, a):
    """Shard + preprocess full inputs into per-core input maps."""
    import ml_dtypes
    c = cfg
    D, K, E = c.d, c.k, c.e
    a = np.asarray(a, np.float32).reshape(-1)
    a_self, a_nb, a_edge = a[:D], a[D:2 * D], a[2 * D:]
    W = np.asarray(W, np.float32)

    # orthogonal Q with column D-1 = a_nb/|a_nb|; S scales that column
    # back to a_nb so y[:, D-1] = x @ a_nb exactly.
    nrm = float(np.linalg.norm(a_nb))
    rng = np.random.default_rng(0)
    base = rng.standard_normal((D, D))
    base[:, 0] = a_nb / nrm
    Qf, _ = np.linalg.qr(base)
    if np.dot(Qf[:, 0], a_nb) < 0:
        Qf[:, 0] *= -1.0
    Q = np.concatenate([Qf[:, 1:], Qf[:, :1]], axis=1)  # col D-1 = a_nb/nrm
    dscale = np.ones(D, np.float32)
    dscale[D - 1] = nrm
    WQ = (W @ Q) * dscale[None, :]
    ws = W @ a_self
    wext = np.concatenate(
        [WQ, W, ((1.0 - 2 * ALPHA) * ws)[:, None],
         (2 * ALPHA * ws)[:, None]], axis=1)
    wext = np.ascontiguousarray(wext, np.float32)
    # un-rotation matrix: h[d] = sum_y M[y, d] * h~[y]
    m_unrot = np.ascontiguousarray(
        (Q / dscale[None, :]).T.astype(ml_dtypes.bfloat16))

    aer = np.ascontiguousarray(
        np.broadcast_to(np.tile(a_edge, K)[None, :], (128, K * E))
        .astype(ml_dtypes.bfloat16))
    # mask[p, 4g+m] = (p // 32 == m)
    pidx, cidx = np.meshgrid(np.arange(128), np.arange(128), indexing="ij")
    msk_m = ((pidx // K) == (cidx % c.nsub)).astype(ml_dtypes.bfloat16)

    # node u -> (table pair row, parity); table rows = per-core shards concat
    neigh = np.asarray(neigh)
    cidx_n = neigh // c.shard
    j = neigh % c.shard
    lp = j // 2
    par_full = (j & 1).astype(np.float32)
    rowidx = cidx_n * c.pairs + lp
    assert rowidx.max() < c.tbl_pairs

    features = np.asarray(features, np.float32)
    emb = np.asarray(emb, np.float32)

    in_maps = []
    for ci in range(c.ncores):
        s0, s1 = ci * c.shard, (ci + 1) * c.shard
        pad = c.shard_pad - c.shard
        f = features[s0:s1]
        if pad:
            f = np.concatenate([f, np.zeros((pad, c.in_dim), np.float32)])
        # host-transposed features: featT[p, (t, c, n)] = f[t*128+n, c*128+p]
        featT = np.ascontiguousarray(
            f.reshape(c.tiles, 128, c.in_dim // 128, 128)
            .transpose(3, 0, 2, 1).reshape(128, c.tiles * c.in_dim))

        em = emb[s0:s1]
        if pad:
            em = np.concatenate([em, np.zeros((pad, K, E), np.float32)])
        # packed emb: embp[t, 32*nsub+k, g*64+e] = em[t, 32*nsub+g, k, e]
        embp = (em.reshape(c.tiles, c.nsub, K, K, E)
                .transpose(0, 1, 3, 2, 4)
                .reshape(c.tiles * 128, K * E).astype(ml_dtypes.bfloat16))

        nr = rowidx[s0:s1]
        pr = par_full[s0:s1]
        if pad:
            nr = np.concatenate([nr, np.zeros((pad, K), nr.dtype)])
            pr = np.concatenate([pr, np.zeros((pad, K), np.float32)])
        # gather stream per tile: pos i = g*128 + (32*nsub + k) holds edge
        # (node 32*nsub + g, neighbor k); block column g = node % 32
        st = (nr.reshape(c.tiles, c.nsub, K, K)    # [t, nsub, g, k]
              .transpose(0, 2, 1, 3)               # [t, g, nsub, k]
              .reshape(c.tiles, c.per_tile_idx)).astype(np.int16)
        # int16 stream wrap per gather call: idx i -> (partition i%16, i//16)
        segs = []
        for cc in range(c.gather_calls):
            seg = st[:, cc * c.chunk:(cc + 1) * c.chunk]
            segs.append(seg.reshape(c.tiles, c.chunk // 16, 16)
                        .transpose(0, 2, 1))       # [t, 16, chunk//16]
        wrapped = np.concatenate(segs, axis=2)     # [t, 16, idx_cols]
        idx16 = np.ascontiguousarray(
            np.tile(wrapped, (1, 8, 1))            # replicate to 128 parts
            .reshape(c.tiles * 128, c.idx_cols))
        # parity in packed layout [p, g]: pos i -> (p=i%128, g=i//128)
        stp = (pr.reshape(c.tiles, c.nsub, K, K)
               .transpose(0, 2, 1, 3).reshape(c.tiles, K, 128))
        par_pk = stp.transpose(0, 2, 1)            # [t, 128, g]
        parr = np.concatenate([par_pk, 1.0 - par_pk], axis=2)
        parr = np.ascontiguousarray(
            parr.reshape(c.tiles * 128, 2 * K).astype(ml_dtypes.bfloat16))
        in_maps.append({
            "featT": featT,
            "wext": wext,
            "embd": np.ascontiguousarray(embp),
            "aer": aer,
            "msk": msk_m,
            "unrot": m_unrot,
            "idx": idx16,
            "parp": parr,
        })
    return in_maps


_CACHE = {}


def _get_compiled(key="full"):
    if key not in _CACHE:
        cfg = Cfg()
        _CACHE[key] = (cfg, build(cfg))
    return _CACHE[key]


def run(inputs, trace=False):
    """Run on hardware. Returns (out [N, 2D+E] f32, exec_time_ns or None)."""
    cfg, nc = _get_compiled()
    in_maps = prep_inputs(cfg, inputs["features"], inputs["neigh"],
                          inputs["emb"], inputs["W"], inputs["a"])
    res = run_bass_kernel_spmd(nc, in_maps, list(range(cfg.ncores)),
                               trace=trace)
    outs = [res.results[ci]["outd"][:cfg.shard] for ci in range(cfg.ncores)]
    out = np.concatenate(outs, axis=0)
    return out, res.exec_time_ns


def kernel(**inputs):
    out, _ = run(inputs)
    return out
